# revision 1
# baseline (speedup 1.0000x reference)
"""LPKT knowledge-tracing kernel for 8x Trainium2 NeuronCores.

Data-parallel over batch: B=32 -> 4 batches per core. Per core the recurrent
state h [4, C=256, K=128] is kept in SBUF transposed as hT [K=128 partitions,
(b,c)=1024 free].  Per step:
  - gates LG from h_tilde via small PE matmuls + sigmoid (tanh folded:
    (tanh(x)+1)/2 == sigmoid(2x))
  - gamma_f preact = W4a^T-matmul over hT (PE), per-batch bias u via
    activation bias operand
  - h_new = q_e (x) LG + gamma_f * h via fused scalar_tensor_tensor with the
    q_e replication (PE rank-1 matmul from previous step) in PSUM
  - h_tilde = sum_c q_next * h_new via scalar_tensor_tensor accum_out
  - y_t accumulated into a PSUM row via ones-matmul (partition reduction)
"""

import numpy as np

B, S = 32, 128
NUM_Q, NUM_C = 10000, 256
K = 128
C = NUM_C
NCORES = 8
BL = B // NCORES  # 4 batches per core
T = S - 1  # 127 recurrence steps
QP = 32  # partitions used for the q table layout

_cache = {}


def _build(np_inputs_shapes_only=None):
    import concourse.bass as bass  # noqa: F401
    import concourse.mybir as mybir
    import concourse.tile as tile
    from concourse import bacc

    fp32 = mybir.dt.float32
    AF = mybir.ActivationFunctionType
    OP = mybir.AluOpType

    nc = bacc.Bacc()

    # ---------------- DRAM I/O ----------------
    def din(name, shape):
        return nc.dram_tensor(name, shape, fp32, kind="ExternalInput")

    d = {}
    d["eT"] = din("eT", [K, BL, S])        # e_emb gathered+transposed
    d["atT"] = din("atT", [K, BL, S])
    d["itT"] = din("itT", [K, BL, S])
    d["qA"] = None  # q rows live in DRAM, staged per step
    q_dram = nc.dram_tensor("qD", [S, BL * C], fp32, kind="ExternalInput")
    del d["qA"]
    d["a_row"] = din("a_row", [1, BL * S])
    d["h0T4"] = din("h0T4", [K, BL * C])
    for w in ["W1a", "W1b", "W2a", "W2b", "W2c", "W2d",
              "W3a", "W3b", "W3c", "W3d", "W4a", "W4b", "W4c",
              "W5a", "W5b"]:
        d[w] = din(w, [K, K])
    for w in ["w1c", "b1r", "b2r", "b3r", "b4r", "b5r", "ones1r"]:
        d[w] = din(w, [1, K])
    d["ones512"] = din("ones512", [1, 512])
    d["ones128c"] = din("ones128c", [K, 1])
    y_dram = nc.dram_tensor("y_out", [1, BL * T], fp32, kind="ExternalOutput")

    from contextlib import ExitStack

    with tile.TileContext(nc) as tc, ExitStack() as ctx:
        singles = ctx.enter_context(tc.tile_pool(name="singles", bufs=1))
        state = ctx.enter_context(tc.tile_pool(name="state", bufs=1))
        sm = ctx.enter_context(tc.tile_pool(name="sm", bufs=4))
        htp = ctx.enter_context(tc.tile_pool(name="htp", bufs=3))
        pp = ctx.enter_context(tc.tile_pool(name="pp", bufs=2, space="PSUM"))
        pq = ctx.enter_context(tc.tile_pool(name="pq", bufs=2, space="PSUM"))
        psm = ctx.enter_context(tc.tile_pool(name="psm", bufs=1, space="PSUM"))
        pyacc = ctx.enter_context(tc.tile_pool(name="pyacc", bufs=1, space="PSUM"))

        # ---------------- load everything to SBUF ----------------
        sb = {}
        for name, dt_ in d.items():
            t_ = singles.tile(list(dt_.shape), fp32, tag=name)
            nc.sync.dma_start(out=t_[:], in_=dt_[:])
            sb[name] = t_

        # collapse the ~30 input-DMA dependencies so no matmul needs >1 wait
        tc.strict_bb_all_engine_barrier()

        s_hT = state.tile([K, BL * C], fp32, tag="hT")
        nc.vector.tensor_copy(out=s_hT[:], in_=sb["h0T4"][:])

        s_gam = state.tile([K, BL * C], fp32, tag="gam")
        s_m = state.tile([K, BL * C], fp32, tag="m")

        # ---------------- precompute: allT, Z2, Z3, U4, Y5 ----------------
        # allT[k, b, s] = (all_learning).T
        p_all = pp.tile([K, BL, S], fp32, tag="pbig")
        nc.tensor.matmul(out=p_all[:], lhsT=sb["W1a"][:], rhs=sb["eT"][:],
                         start=True, stop=False)
        nc.tensor.matmul(out=p_all[:], lhsT=sb["W1b"][:], rhs=sb["atT"][:],
                         start=False, stop=False)
        nc.tensor.matmul(out=p_all[:], lhsT=sb["w1c"][:], rhs=sb["a_row"][:],
                         start=False, stop=False)
        nc.tensor.matmul(out=p_all[:], lhsT=sb["b1r"][:],
                         rhs=sb["ones512"][:, 0:512], start=False, stop=True)
        s_allT = singles.tile([K, BL, S], fp32, tag="allT")
        nc.vector.tensor_copy(out=s_allT[:], in_=p_all[:])

        def precompute_z(Wpre, Wit, Wlearn, brow, tag):
            # out[:, b, t] = lp[t]@Wpre + it[t]@Wit + learn[t]@Wlearn + b
            ptile = pp.tile([K, BL, T], fp32, tag="pbig")
            nc.tensor.matmul(out=ptile[:], lhsT=sb[Wit][:],
                             rhs=sb["itT"][:, :, 0:T], start=True, stop=False)
            if Wpre is not None:
                nc.tensor.matmul(out=ptile[:, :, 1:T], lhsT=sb[Wpre][:],
                                 rhs=sb["allT"][:, :, 0:T - 1],
                                 start=False, stop=False, skip_group_check=True)
            if Wlearn is not None:
                nc.tensor.matmul(out=ptile[:], lhsT=sb[Wlearn][:],
                                 rhs=sb["allT"][:, :, 0:T], start=False, stop=False)
            nc.tensor.matmul(out=ptile[:], lhsT=sb[brow][:],
                             rhs=sb["ones512"][:, 0:BL * T], start=False, stop=True)
            s = singles.tile([K, BL, T], fp32, tag=tag)
            nc.vector.tensor_copy(out=s[:], in_=ptile[:])
            return s

        sb["allT"] = s_allT
        s_Z2 = precompute_z("W2a", "W2b", "W2c", "b2r", "Z2")
        s_Z3 = precompute_z("W3a", "W3b", "W3c", "b3r", "Z3")

        # U4[:, b, t] = it[t] @ W4c + b4
        p_u4 = pp.tile([K, BL, T], fp32, tag="pbig")
        nc.tensor.matmul(out=p_u4[:], lhsT=sb["W4c"][:],
                         rhs=sb["itT"][:, :, 0:T], start=True, stop=False)
        nc.tensor.matmul(out=p_u4[:], lhsT=sb["b4r"][:],
                         rhs=sb["ones512"][:, 0:BL * T], start=False, stop=True)
        s_U4 = singles.tile([K, BL, T], fp32, tag="U4")
        nc.vector.tensor_copy(out=s_U4[:], in_=p_u4[:])

        # Y5[:, b, t] = e_emb[t+1] @ W5a + b5
        p_y5 = pp.tile([K, BL, T], fp32, tag="pbig")
        nc.tensor.matmul(out=p_y5[:], lhsT=sb["W5a"][:],
                         rhs=sb["eT"][:, :, 1:S], start=True, stop=False)
        nc.tensor.matmul(out=p_y5[:], lhsT=sb["b5r"][:],
                         rhs=sb["ones512"][:, 0:BL * T], start=False, stop=True)
        s_Y5 = singles.tile([K, BL, T], fp32, tag="Y5")
        nc.vector.tensor_copy(out=s_Y5[:], in_=p_y5[:])

        # ---------------- q staging + replication helpers ----------------
        qstage = [None] * S

        def qstage_load(t):
            st = sm.tile([1, BL * C], fp32, tag="qstage", bufs=6)
            nc.sync.dma_start(out=st[:], in_=q_dram[t:t + 1, :])
            qstage[t] = st

        def qrep(t):
            # replicate q_t rows for all 4 batches across 128 partitions:
            # two [128, 512] psum tiles (batches 0,1 then 2,3)
            qt0 = pq.tile([K, 512], fp32, tag="q0")
            qt1 = pq.tile([K, 512], fp32, tag="q1")
            st = qstage[t]
            nc.tensor.matmul(out=qt0[:], lhsT=sb["ones1r"][:],
                             rhs=st[:, 0:512], start=True, stop=True)
            nc.tensor.matmul(out=qt1[:], lhsT=sb["ones1r"][:],
                             rhs=st[:, 512:1024], start=True, stop=True)
            return (qt0, qt1)

        def qhalf(qpair, b):
            # [128, 256] slice of the replicated q for batch b
            return qpair[b // 2][:, (b % 2) * C:(b % 2 + 1) * C]

        # ---------------- h_tilde init (with q_0) ----------------
        qstage_load(0)
        qstage_load(1)
        q_prev = qrep(0)
        ht_prev = htp.tile([K, BL], fp32, tag="ht")
        for b in range(BL):
            nc.vector.scalar_tensor_tensor(
                out=s_gam[:, 0:C], in0=s_hT[:, b * C:(b + 1) * C], scalar=0.0,
                in1=qhalf(q_prev, b), op0=OP.bypass, op1=OP.mult,
                accum_out=ht_prev[:, b:b + 1])

        p_y = pyacc.tile([1, BL * T], fp32, tag="yacc")

        # ---------------- the recurrence ----------------
        for t in range(T):
            if t + 2 < S:
                qstage_load(t + 2)
            ps = psm.tile([K, 16], fp32, tag="small")
            # gates: lg/gamma_l preacts from h_tilde
            nc.tensor.matmul(out=ps[:, 0:4], lhsT=sb["W2d"][:], rhs=ht_prev[:],
                             start=True, stop=True)
            nc.tensor.matmul(out=ps[:, 4:8], lhsT=sb["W3d"][:], rhs=ht_prev[:],
                             start=True, stop=True)
            tA = sm.tile([K, BL], fp32, tag="tA")
            tB = sm.tile([K, BL], fp32, tag="tB")
            nc.vector.tensor_add(out=tA[:], in0=ps[:, 0:4], in1=s_Z2[:, :, t])
            nc.vector.tensor_add(out=tB[:], in0=ps[:, 4:8], in1=s_Z3[:, :, t])
            s2 = sm.tile([K, BL], fp32, tag="s2")
            s3 = sm.tile([K, BL], fp32, tag="s3")
            nc.scalar.activation(out=s2[:], in_=tA[:], func=AF.Sigmoid, scale=2.0)
            nc.scalar.activation(out=s3[:], in_=tB[:], func=AF.Sigmoid)
            LGT = sm.tile([K, BL], fp32, tag="LGT")
            nc.vector.tensor_mul(out=LGT[:], in0=s2[:], in1=s3[:])

            # u = LG @ W4b + U4[t]
            nc.tensor.matmul(out=ps[:, 8:12], lhsT=sb["W4b"][:], rhs=LGT[:],
                             start=True, stop=True)
            uT = sm.tile([K, BL], fp32, tag="uT")
            nc.vector.tensor_add(out=uT[:], in0=ps[:, 8:12], in1=s_U4[:, :, t])

            # gamma_f preact (big matmul over hT)
            pP0 = pp.tile([K, 512], fp32, tag="pbig")
            pP1 = pp.tile([K, 512], fp32, tag="pbig")
            nc.tensor.matmul(out=pP0[:], lhsT=sb["W4a"][:], rhs=s_hT[:, 0:512],
                             start=True, stop=True)
            nc.tensor.matmul(out=pP1[:], lhsT=sb["W4a"][:], rhs=s_hT[:, 512:1024],
                             start=True, stop=True)
            for b in range(BL):
                src = (pP0 if b < 2 else pP1)[:, (b % 2) * C:(b % 2 + 1) * C]
                nc.scalar.activation(out=s_gam[:, b * C:(b + 1) * C], in_=src,
                                     func=AF.Sigmoid, bias=uT[:, b:b + 1])

            # m = gamma * h ; h_new = q_e_rep * LG + m   (q_e_rep from prev step)
            nc.vector.tensor_mul(out=s_m[:], in0=s_gam[:], in1=s_hT[:])
            for b in range(BL):
                nc.vector.scalar_tensor_tensor(
                    out=s_hT[:, b * C:(b + 1) * C], in0=qhalf(q_prev, b),
                    scalar=LGT[:, b:b + 1], in1=s_m[:, b * C:(b + 1) * C],
                    op0=OP.mult, op1=OP.add)

            # replicate q_{t+1}; h_tilde_new = sum_c q_next * h_new
            q_next = qrep(t + 1)
            ht_new = htp.tile([K, BL], fp32, tag="ht")
            for b in range(BL):
                nc.vector.scalar_tensor_tensor(
                    out=s_gam[:, b * C:(b + 1) * C],
                    in0=s_hT[:, b * C:(b + 1) * C], scalar=0.0,
                    in1=qhalf(q_next, b), op0=OP.bypass, op1=OP.mult,
                    accum_out=ht_new[:, b:b + 1])

            # y_t = sigmoid(e_next@W5a + h_tilde@W5b + b5) summed over K
            nc.tensor.matmul(out=ps[:, 12:16], lhsT=sb["W5b"][:], rhs=ht_new[:],
                             start=True, stop=True)
            tY = sm.tile([K, BL], fp32, tag="tY")
            nc.vector.tensor_add(out=tY[:], in0=ps[:, 12:16], in1=s_Y5[:, :, t])
            sY = sm.tile([K, BL], fp32, tag="sY")
            nc.scalar.activation(out=sY[:], in_=tY[:], func=AF.Sigmoid)
            nc.tensor.matmul(out=p_y[0:1, 4 * t:4 * t + 4], lhsT=sb["ones128c"][:],
                             rhs=sY[:], start=True, stop=True)

            q_prev = q_next
            ht_prev = ht_new

        s_y = singles.tile([1, BL * T], fp32, tag="yout")
        nc.vector.tensor_copy(out=s_y[:], in_=p_y[:])
        nc.sync.dma_start(out=y_dram[:], in_=s_y[:])

    nc.compile()
    return nc


def _prep_inputs(inputs):
    """Host-side sharding + layout prep. Returns per-core input dicts."""
    f32 = np.float32
    e_idx = np.asarray(inputs["e_data"]).astype(np.int64)
    at_idx = np.asarray(inputs["at_data"]).astype(np.int64)
    it_idx = np.asarray(inputs["it_data"]).astype(np.int64)
    a_data = np.asarray(inputs["a_data"], dtype=f32)
    q_matrix = np.asarray(inputs["q_matrix"], dtype=f32)
    e_E = np.asarray(inputs["e_E"], dtype=f32)
    at_E = np.asarray(inputs["at_E"], dtype=f32)
    it_E = np.asarray(inputs["it_E"], dtype=f32)
    W1 = np.asarray(inputs["W1"], dtype=f32)
    W2 = np.asarray(inputs["W2"], dtype=f32)
    W3 = np.asarray(inputs["W3"], dtype=f32)
    W4 = np.asarray(inputs["W4"], dtype=f32)
    W5 = np.asarray(inputs["W5"], dtype=f32)
    h0 = np.asarray(inputs["h0"], dtype=f32)

    shared = {
        "W1a": W1[0:K], "W1b": W1[K:2 * K],
        "w1c": W1[2 * K:].sum(0)[None, :] .astype(f32),
        "b1r": np.asarray(inputs["b1"], dtype=f32)[None, :],
        "W2a": W2[0:K], "W2b": W2[K:2 * K], "W2c": W2[2 * K:3 * K], "W2d": W2[3 * K:],
        "b2r": np.asarray(inputs["b2"], dtype=f32)[None, :],
        "W3a": W3[0:K], "W3b": W3[K:2 * K], "W3c": W3[2 * K:3 * K], "W3d": W3[3 * K:],
        "b3r": np.asarray(inputs["b3"], dtype=f32)[None, :],
        "W4a": W4[0:K], "W4b": W4[K:2 * K], "W4c": W4[2 * K:],
        "b4r": np.asarray(inputs["b4"], dtype=f32)[None, :],
        "W5a": W5[0:K], "W5b": W5[K:],
        "b5r": np.asarray(inputs["b5"], dtype=f32)[None, :],
        "ones1r": np.ones((1, K), f32),
        "ones512": np.ones((1, 512), f32),
        "ones128c": np.ones((K, 1), f32),
        "h0T4": np.tile(np.ascontiguousarray(h0.T), (1, BL)),
    }

    in_maps = []
    for g in range(NCORES):
        bg = slice(g * BL, (g + 1) * BL)
        e_emb = e_E[e_idx[bg]]          # [4, S, K]
        at_emb = at_E[at_idx[bg]]
        it_emb = it_E[it_idx[bg]]
        q_all = q_matrix[e_idx[bg]]     # [4, S, C]
        # qD[t, b*256 + c] = q_all[b, t, c]
        qD = np.ascontiguousarray(q_all.transpose(1, 0, 2).reshape(S, BL * C))
        m = dict(shared)
        m["eT"] = np.ascontiguousarray(e_emb.reshape(BL * S, K).T).reshape(K, BL, S)
        m["atT"] = np.ascontiguousarray(at_emb.reshape(BL * S, K).T).reshape(K, BL, S)
        m["itT"] = np.ascontiguousarray(it_emb.reshape(BL * S, K).T).reshape(K, BL, S)
        m["qD"] = qD
        m["a_row"] = np.ascontiguousarray(a_data[bg].reshape(1, BL * S))
        in_maps.append({k: np.ascontiguousarray(v) for k, v in m.items()})
    return in_maps


def _run(inputs, trace=False):
    from concourse.bass_utils import run_bass_kernel_spmd

    nc = _build()
    in_maps = _prep_inputs(inputs)
    res = run_bass_kernel_spmd(nc, in_maps, core_ids=list(range(NCORES)),
                               trace=trace)
    pred = np.zeros((B, S), np.float32)
    for g in range(NCORES):
        y = res.results[g]["y_out"].reshape(T, BL)  # [t, b]
        pred[g * BL:(g + 1) * BL, 1:] = y.T / K
    return pred, res


def kernel(**inputs):
    return _run(inputs)[0]



# revision 4
# speedup vs baseline: 1.8570x; 1.8570x over previous
"""LPKT knowledge-tracing kernel for 8x Trainium2 NeuronCores.

Data-parallel over batch: B=32 -> 4 batches per core. Per core the recurrent
state h [4, C=256, K=128] lives in SBUF as hT [K=128 partitions, (b,c)=1024
free] in bf16.  All matmuls are bf16 (1 HW pass, 1 cycle/row vs fp32's 2
passes at 4 cycles/row), all big elementwise ops are bf16 SBUF-only
scalar_tensor_tensor (DVE 4x mode).  q-row replication across partitions is
done on the (otherwise idle) GpSimd engine via partition_broadcast,
prefetched 2 steps ahead, so no PE rank-1 matmuls and no PSUM operands in
the DVE inner loop.  h_tilde columns accumulate into a persistent HT_all
buffer [K, 4*(T+1)]; the per-step gate matmuls read their [K,4] slice
directly and the y head (W5 + sigmoid + reduce) runs once, batched, after
the loop.

Per step the serial chain is:
  ht -> PE (g2,g3 gate matmuls) -> DVE (add Z23) -> ACT (one sigmoid [K,8],
  tanh folded via 2x-scaled W2) -> DVE (LG mult) -> PE (W4b matmul) ->
  DVE (add U4) -> ACT (gamma_f sigmoid x4, bias=u per batch, reading the
  W4a@h PSUM which PE filled concurrently) -> DVE (m, h_new, h_tilde
  accumulation per batch, pipelined behind the 4 sigmoids).
"""

import numpy as np

B, S = 32, 128
NUM_Q, NUM_C = 10000, 256
K = 128
C = NUM_C
NCORES = 8
BL = B // NCORES  # 4 batches per core
T = S - 1  # 127 recurrence steps

_cache = {}


def _build():
    import concourse.bass as bass  # noqa: F401
    import concourse.mybir as mybir
    import concourse.tile as tile
    from concourse import bacc

    fp32 = mybir.dt.float32
    bf16 = mybir.dt.bfloat16
    AF = mybir.ActivationFunctionType
    OP = mybir.AluOpType

    nc = bacc.Bacc()

    # ---------------- DRAM I/O ----------------
    d = {}

    def din(name, shape, dt_=bf16):
        t = nc.dram_tensor(name, shape, dt_, kind="ExternalInput")
        d[name] = t
        return t

    din("eT", [K, S * BL])       # e_emb^T, free layout (s, b) s-major
    din("atT", [K, S * BL])
    din("itT", [K, S * BL])
    din("a_row", [1, S * BL])
    din("h0T4", [K, BL * C])
    q_dram = nc.dram_tensor("qD", [S, BL * C], bf16, kind="ExternalInput")
    for w in ["W1a", "W1b", "W2a2", "W2b2", "W2c2", "W2d2",
              "W3a", "W3b", "W3c", "W3d", "W4a", "W4b", "W4c",
              "W5a", "W5b"]:
        din(w, [K, K])
    for w in ["w1c", "b1r", "b2r2", "b3r", "b4r", "b5r"]:
        din(w, [1, K])
    din("ones512", [1, 512])
    din("ones128c", [K, 1])
    y_dram = nc.dram_tensor("y_out", [1, BL * T], fp32, kind="ExternalOutput")

    from contextlib import ExitStack

    with tile.TileContext(nc) as tc, ExitStack() as ctx:
        singles = ctx.enter_context(tc.tile_pool(name="singles", bufs=1))
        state = ctx.enter_context(tc.tile_pool(name="state", bufs=1))
        sm = ctx.enter_context(tc.tile_pool(name="sm", bufs=3))
        qs = ctx.enter_context(tc.tile_pool(name="qs", bufs=6))
        qr = ctx.enter_context(tc.tile_pool(name="qr", bufs=4))
        pp = ctx.enter_context(tc.tile_pool(name="pp", bufs=2, space="PSUM"))
        psm = ctx.enter_context(tc.tile_pool(name="psm", bufs=2, space="PSUM"))

        # ---------------- load everything to SBUF ----------------
        sb = {}
        for name, dt_ in d.items():
            if name == "h0T4":
                continue  # loaded straight into the state tile below
            t_ = singles.tile(list(dt_.shape), dt_.dtype, tag=name)
            nc.sync.dma_start(out=t_[:], in_=dt_[:])
            sb[name] = t_

        # recurrent state h, DMA'd straight from the prepped h0 tile
        s_h = state.tile([K, BL * C], bf16, tag="h")
        nc.sync.dma_start(out=s_h[:], in_=d["h0T4"][:])

        # stage the first q rows
        qstage = [None] * S

        def qstage_load(t):
            st = qs.tile([1, BL * C], bf16, tag="qstage")
            nc.sync.dma_start(out=st[:], in_=q_dram[t:t + 1, :])
            qstage[t] = st

        for t0 in range(3):
            qstage_load(t0)

        # collapse the ~30 input-DMA dependencies
        tc.strict_bb_all_engine_barrier()

        s_gam = state.tile([K, BL * C], bf16, tag="gam")
        s_m = state.tile([K, BL * C], bf16, tag="m")
        # h_tilde history: block t (cols 4t:4t+4) = h_tilde at step t
        s_HT = state.tile([K, (T + 1) * BL], bf16, tag="HT")

        # ---------------- precompute: allT, Z23, U4, Y5 ----------------
        p_all = pp.tile([K, 512], fp32, tag="pbig", bufs=1)
        nc.tensor.matmul(out=p_all[:], lhsT=sb["W1a"][:], rhs=sb["eT"][:],
                         start=True, stop=False)
        nc.tensor.matmul(out=p_all[:], lhsT=sb["W1b"][:], rhs=sb["atT"][:],
                         start=False, stop=False)
        nc.tensor.matmul(out=p_all[:], lhsT=sb["w1c"][:], rhs=sb["a_row"][:],
                         start=False, stop=False)
        nc.tensor.matmul(out=p_all[:], lhsT=sb["b1r"][:],
                         rhs=sb["ones512"][:], start=False, stop=True)
        s_allT = singles.tile([K, 512], bf16, tag="allT")
        nc.vector.tensor_copy(out=s_allT[:], in_=p_all[:])

        # Z23[k, t, (gate,b)]: gate2 cols 0:4 (2x-scaled), gate3 cols 4:8
        s_Z23 = singles.tile([K, T, 8], fp32, tag="Z23")

        def precompute_z(Wpre, Wit, Wlearn, brow, gslice):
            ptile = pp.tile([K, T * BL], fp32, tag="pbig", bufs=1)
            nc.tensor.matmul(out=ptile[:], lhsT=sb[Wit][:],
                             rhs=sb["itT"][:, 0:T * BL], start=True, stop=False)
            nc.tensor.matmul(out=ptile[:, BL:T * BL], lhsT=sb[Wpre][:],
                             rhs=s_allT[:, 0:(T - 1) * BL],
                             start=False, stop=False, skip_group_check=True)
            nc.tensor.matmul(out=ptile[:], lhsT=sb[Wlearn][:],
                             rhs=s_allT[:, 0:T * BL], start=False, stop=False)
            nc.tensor.matmul(out=ptile[:], lhsT=sb[brow][:],
                             rhs=sb["ones512"][:, 0:T * BL], start=False,
                             stop=True)
            nc.vector.tensor_copy(out=s_Z23[:, :, gslice],
                                  in_=ptile[:].rearrange("k (t b) -> k t b", b=BL))

        precompute_z("W2a2", "W2b2", "W2c2", "b2r2", slice(0, 4))
        precompute_z("W3a", "W3b", "W3c", "b3r", slice(4, 8))

        # U4[k, (t,b)] = it@W4c + b4
        p_u4 = pp.tile([K, T * BL], fp32, tag="pbig", bufs=1)
        nc.tensor.matmul(out=p_u4[:], lhsT=sb["W4c"][:],
                         rhs=sb["itT"][:, 0:T * BL], start=True, stop=False)
        nc.tensor.matmul(out=p_u4[:], lhsT=sb["b4r"][:],
                         rhs=sb["ones512"][:, 0:T * BL], start=False, stop=True)
        s_U4 = singles.tile([K, T * BL], fp32, tag="U4")
        nc.vector.tensor_copy(out=s_U4[:], in_=p_u4[:])

        # Y5[k, (t,b)] = e_emb[t+1]@W5a + b5
        p_y5 = pp.tile([K, T * BL], fp32, tag="pbig", bufs=1)
        nc.tensor.matmul(out=p_y5[:], lhsT=sb["W5a"][:],
                         rhs=sb["eT"][:, BL:S * BL], start=True, stop=False)
        nc.tensor.matmul(out=p_y5[:], lhsT=sb["b5r"][:],
                         rhs=sb["ones512"][:, 0:T * BL], start=False, stop=True)
        s_Y5 = singles.tile([K, T * BL], fp32, tag="Y5")
        nc.vector.tensor_copy(out=s_Y5[:], in_=p_y5[:])

        # ---------------- q replication (GpSimd) ----------------
        qrep = [None] * S

        def qrep_make(t):
            rt = qr.tile([K, BL * C], bf16, tag="qrep")
            nc.gpsimd.partition_broadcast(rt[:], qstage[t][:])
            qrep[t] = rt

        qrep_make(0)
        qrep_make(1)

        # ---------------- h_tilde init (with q_0) ----------------
        for b in range(BL):
            cs = slice(b * C, (b + 1) * C)
            nc.vector.scalar_tensor_tensor(
                out=s_m[:, cs], in0=s_h[:, cs], scalar=0.0,
                in1=qrep[0][:, cs], op0=OP.bypass, op1=OP.mult,
                accum_out=s_HT[:, b:b + 1])

        # ---------------- the recurrence ----------------
        for t in range(T):
            if t + 3 < S:
                qstage_load(t + 3)
            if t + 2 < S:
                qrep_make(t + 2)

            ps = psm.tile([K, 16], fp32, tag="small")
            # gate preacts from h_tilde (rhs is the HT_all slice, bf16)
            ht_sl = s_HT[:, t * BL:(t + 1) * BL]
            nc.tensor.matmul(out=ps[:, 0:4], lhsT=sb["W2d2"][:], rhs=ht_sl,
                             start=True, stop=True)
            nc.tensor.matmul(out=ps[:, 4:8], lhsT=sb["W3d"][:], rhs=ht_sl,
                             start=True, stop=True)
            # gamma_f preact: W4a @ h, fills PE while the gate chain runs
            pP0 = pp.tile([K, 512], fp32, tag="pP0")
            pP1 = pp.tile([K, 512], fp32, tag="pP1")
            nc.tensor.matmul(out=pP0[:], lhsT=sb["W4a"][:], rhs=s_h[:, 0:512],
                             start=True, stop=True)
            nc.tensor.matmul(out=pP1[:], lhsT=sb["W4a"][:], rhs=s_h[:, 512:1024],
                             start=True, stop=True)

            tAB = sm.tile([K, 8], fp32, tag="tAB")
            nc.vector.tensor_add(out=tAB[:], in0=ps[:, 0:8], in1=s_Z23[:, t, :])
            s23 = sm.tile([K, 8], bf16, tag="s23")
            nc.scalar.activation(out=s23[:], in_=tAB[:], func=AF.Sigmoid)
            LGT = sm.tile([K, BL], bf16, tag="LGT")
            nc.vector.tensor_mul(out=LGT[:], in0=s23[:, 0:4], in1=s23[:, 4:8])

            # u = LG @ W4b + U4[t]
            nc.tensor.matmul(out=ps[:, 8:12], lhsT=sb["W4b"][:], rhs=LGT[:],
                             start=True, stop=True)
            uT = sm.tile([K, BL], fp32, tag="uT")
            nc.vector.tensor_add(out=uT[:], in0=ps[:, 8:12],
                                 in1=s_U4[:, t * BL:(t + 1) * BL])

            # per batch: gamma sigmoid, then m / h_new / h_tilde accumulation
            for b in range(BL):
                cs = slice(b * C, (b + 1) * C)
                src = (pP0 if b < 2 else pP1)[:, (b % 2) * C:(b % 2 + 1) * C]
                nc.scalar.activation(out=s_gam[:, cs], in_=src,
                                     func=AF.Sigmoid, bias=uT[:, b:b + 1])
                nc.vector.scalar_tensor_tensor(
                    out=s_m[:, cs], in0=s_gam[:, cs], scalar=0.0,
                    in1=s_h[:, cs], op0=OP.bypass, op1=OP.mult)
                nc.vector.scalar_tensor_tensor(
                    out=s_h[:, cs], in0=qrep[t][:, cs], scalar=LGT[:, b:b + 1],
                    in1=s_m[:, cs], op0=OP.mult, op1=OP.add)
                nc.vector.scalar_tensor_tensor(
                    out=s_m[:, cs], in0=s_h[:, cs], scalar=0.0,
                    in1=qrep[t + 1][:, cs], op0=OP.bypass, op1=OP.mult,
                    accum_out=s_HT[:, (t + 1) * BL + b:(t + 1) * BL + b + 1])

        # ---------------- y head, batched over all steps ----------------
        p_y = pp.tile([K, T * BL], fp32, tag="pbig", bufs=1)
        nc.tensor.matmul(out=p_y[:], lhsT=sb["W5b"][:],
                         rhs=s_HT[:, BL:(T + 1) * BL], start=True, stop=True)
        tY = singles.tile([K, T * BL], fp32, tag="tY")
        nc.vector.tensor_add(out=tY[:], in0=p_y[:], in1=s_Y5[:])
        sY = singles.tile([K, T * BL], bf16, tag="sY")
        nc.scalar.activation(out=sY[:], in_=tY[:], func=AF.Sigmoid)
        p_ys = psm.tile([1, T * BL], fp32, tag="yacc", bufs=1)
        nc.tensor.matmul(out=p_ys[:], lhsT=sb["ones128c"][:], rhs=sY[:],
                         start=True, stop=True)
        s_y = singles.tile([1, T * BL], fp32, tag="yout")
        nc.vector.tensor_copy(out=s_y[:], in_=p_ys[:])
        nc.sync.dma_start(out=y_dram[:], in_=s_y[:])

    nc.compile()
    return nc


def _prep_inputs(inputs):
    """Host-side sharding + layout prep. Returns per-core input dicts."""
    import ml_dtypes

    bf = ml_dtypes.bfloat16
    f32 = np.float32
    e_idx = np.asarray(inputs["e_data"]).astype(np.int64)
    at_idx = np.asarray(inputs["at_data"]).astype(np.int64)
    it_idx = np.asarray(inputs["it_data"]).astype(np.int64)
    a_data = np.asarray(inputs["a_data"], dtype=f32)
    q_matrix = np.asarray(inputs["q_matrix"], dtype=f32)
    e_E = np.asarray(inputs["e_E"], dtype=bf)
    at_E = np.asarray(inputs["at_E"], dtype=bf)
    it_E = np.asarray(inputs["it_E"], dtype=bf)
    W1 = np.asarray(inputs["W1"], dtype=f32)
    W2 = np.asarray(inputs["W2"], dtype=f32)
    W3 = np.asarray(inputs["W3"], dtype=f32)
    W4 = np.asarray(inputs["W4"], dtype=f32)
    W5 = np.asarray(inputs["W5"], dtype=f32)
    h0 = np.asarray(inputs["h0"], dtype=f32)

    def bfc(x):
        return np.ascontiguousarray(np.asarray(x, dtype=bf))

    shared = {
        "W1a": bfc(W1[0:K]), "W1b": bfc(W1[K:2 * K]),
        "w1c": bfc(W1[2 * K:].sum(0)[None, :]),
        "b1r": bfc(np.asarray(inputs["b1"], dtype=f32)[None, :]),
        "W2a2": bfc(2 * W2[0:K]), "W2b2": bfc(2 * W2[K:2 * K]),
        "W2c2": bfc(2 * W2[2 * K:3 * K]), "W2d2": bfc(2 * W2[3 * K:]),
        "b2r2": bfc(2 * np.asarray(inputs["b2"], dtype=f32)[None, :]),
        "W3a": bfc(W3[0:K]), "W3b": bfc(W3[K:2 * K]),
        "W3c": bfc(W3[2 * K:3 * K]), "W3d": bfc(W3[3 * K:]),
        "b3r": bfc(np.asarray(inputs["b3"], dtype=f32)[None, :]),
        "W4a": bfc(W4[0:K]), "W4b": bfc(W4[K:2 * K]), "W4c": bfc(W4[2 * K:]),
        "b4r": bfc(np.asarray(inputs["b4"], dtype=f32)[None, :]),
        "W5a": bfc(W5[0:K]), "W5b": bfc(W5[K:]),
        "b5r": bfc(np.asarray(inputs["b5"], dtype=f32)[None, :]),
        "ones512": bfc(np.ones((1, 512), f32)),
        "ones128c": bfc(np.ones((K, 1), f32)),
        "h0T4": bfc(np.tile(np.ascontiguousarray(h0.T), (1, BL))),
    }

    in_maps = []
    for g in range(NCORES):
        bg = slice(g * BL, (g + 1) * BL)
        e_emb = e_E[e_idx[bg]]          # [4, S, K] bf16
        at_emb = at_E[at_idx[bg]]
        it_emb = it_E[it_idx[bg]]
        q_all = q_matrix[e_idx[bg]]     # [4, S, C] f32
        m = dict(shared)
        # [K, (s, b)] s-major layouts
        m["eT"] = bfc(e_emb.transpose(2, 1, 0).reshape(K, S * BL))
        m["atT"] = bfc(at_emb.transpose(2, 1, 0).reshape(K, S * BL))
        m["itT"] = bfc(it_emb.transpose(2, 1, 0).reshape(K, S * BL))
        m["qD"] = bfc(q_all.transpose(1, 0, 2).reshape(S, BL * C))
        m["a_row"] = bfc(a_data[bg].T.reshape(1, S * BL))
        in_maps.append(m)
    return in_maps


def _run(inputs, trace=False):
    from concourse.bass_utils import run_bass_kernel_spmd

    if "nc" not in _cache:
        _cache["nc"] = _build()
    nc = _cache["nc"]
    in_maps = _prep_inputs(inputs)
    res = run_bass_kernel_spmd(nc, in_maps, core_ids=list(range(NCORES)),
                               trace=trace)
    pred = np.zeros((B, S), np.float32)
    for g in range(NCORES):
        y = res.results[g]["y_out"].reshape(T, BL)  # [t, b]
        pred[g * BL:(g + 1) * BL, 1:] = y.T / K
    return pred, res


def kernel(**inputs):
    return _run(inputs)[0]


# revision 6
# speedup vs baseline: 2.0630x; 1.1109x over previous
"""LPKT knowledge-tracing kernel for 8x Trainium2 NeuronCores.

Data-parallel over batch: B=32 -> 4 batches per core. Per core the recurrent
state h [4, C=256, K=128] lives in SBUF as hT [K=128 partitions, (b,c)=1024
free] in bf16.  All matmuls are bf16; elementwise work is spread across DVE
(h_new, h_tilde accumulation, small adds) and GpSimd (LG gate product and
the gamma*h multiplies).  q rows are broadcast across all 128 partitions by
DMA (stride-0 source), one 16-step window (4MB) at a time, double buffered
-- the descriptors fan out over all 16 DMA engines, so no compute engine
spends cycles on replication.

The 4 batches are processed as TWO independent 2-batch streams per step so
stream B's gate chain (PE matmul -> sigmoid -> LG -> W4b matmul -> u)
overlaps stream A's state-update tail, keeping DVE/Pool busy instead of
serializing the whole step.  h_tilde columns accumulate into a persistent
HT_all buffer [K, 4*(T+1)]; gate matmuls read their [K,2] slice directly
and the y head (W5 + sigmoid + reduce) runs once, batched, after the loop.
"""

import numpy as np

B, S = 32, 128
NUM_Q, NUM_C = 10000, 256
K = 128
C = NUM_C
NCORES = 8
BL = B // NCORES  # 4 batches per core
T = S - 1  # 127 recurrence steps
QW = 16  # q broadcast window, steps
NWIN = S // QW

_cache = {}


def _build():
    import concourse.bass as bass  # noqa: F401
    import concourse.mybir as mybir
    import concourse.tile as tile
    from concourse import bacc

    fp32 = mybir.dt.float32
    bf16 = mybir.dt.bfloat16
    AF = mybir.ActivationFunctionType
    OP = mybir.AluOpType

    nc = bacc.Bacc()

    # ---------------- DRAM I/O ----------------
    d = {}

    def din(name, shape, dt_=bf16):
        t = nc.dram_tensor(name, shape, dt_, kind="ExternalInput")
        d[name] = t
        return t

    din("eT", [K, S * BL])       # e_emb^T, free layout (s, b) s-major
    din("atT", [K, S * BL])
    din("itT", [K, S * BL])
    din("a_row", [1, S * BL])
    din("h0T4", [K, BL * C])
    q_dram = nc.dram_tensor("qD", [S, BL * C], bf16, kind="ExternalInput")
    for w in ["W1a", "W1b", "W2a2", "W2b2", "W2c2", "W2d2",
              "W3a", "W3b", "W3c", "W3d", "W4a", "W4b", "W4c",
              "W5a", "W5b"]:
        din(w, [K, K])
    for w in ["w1c", "b1r", "b2r2", "b3r", "b4r", "b5r"]:
        din(w, [1, K])
    din("ones512", [1, 512])
    din("ones128c", [K, 1])
    y_dram = nc.dram_tensor("y_out", [1, BL * T], fp32, kind="ExternalOutput")

    from contextlib import ExitStack

    with tile.TileContext(nc) as tc, ExitStack() as ctx:
        singles = ctx.enter_context(tc.tile_pool(name="singles", bufs=1))
        state = ctx.enter_context(tc.tile_pool(name="state", bufs=1))
        sm = ctx.enter_context(tc.tile_pool(name="sm", bufs=3))
        qw = ctx.enter_context(tc.tile_pool(name="qw", bufs=2))
        pp = ctx.enter_context(tc.tile_pool(name="pp", bufs=2, space="PSUM"))
        psm = ctx.enter_context(tc.tile_pool(name="psm", bufs=2, space="PSUM"))

        # ---------------- load everything to SBUF ----------------
        sb = {}
        for name, dt_ in d.items():
            if name == "h0T4":
                continue  # loaded straight into the state tile below
            t_ = singles.tile(list(dt_.shape), dt_.dtype, tag=name)
            nc.sync.dma_start(out=t_[:], in_=dt_[:])
            sb[name] = t_

        # recurrent state h, DMA'd straight from the prepped h0 tile
        s_h = state.tile([K, BL * C], bf16, tag="h")
        nc.sync.dma_start(out=s_h[:], in_=d["h0T4"][:])

        # q windows: all 128 partitions get a copy of q rows [16w, 16w+16)
        qwin = [None] * NWIN

        def qwin_load(w):
            wt = qw.tile([K, QW * BL * C], bf16, tag="qwin")
            src = q_dram[w * QW:(w + 1) * QW, :].partition_broadcast(K)
            nc.sync.dma_start(out=wt[:], in_=src)
            qwin[w] = wt

        qwin_load(0)
        qwin_load(1)

        def qsl(t):
            # [K, 1024] replicated q row for step t
            base = (t % QW) * BL * C
            return qwin[t // QW][:, base:base + BL * C]

        # collapse the ~30 input-DMA dependencies
        tc.strict_bb_all_engine_barrier()

        s_gam = state.tile([K, BL * C], bf16, tag="gam")
        s_m = state.tile([K, BL * C], bf16, tag="m")
        # h_tilde history: block t (cols 4t:4t+4) = h_tilde at step t
        s_HT = state.tile([K, (T + 1) * BL], bf16, tag="HT")

        # ---------------- precompute: allT, Z23, U4, Y5 ----------------
        p_all = pp.tile([K, 512], fp32, tag="pbig", bufs=1)
        nc.tensor.matmul(out=p_all[:], lhsT=sb["W1a"][:], rhs=sb["eT"][:],
                         start=True, stop=False)
        nc.tensor.matmul(out=p_all[:], lhsT=sb["W1b"][:], rhs=sb["atT"][:],
                         start=False, stop=False)
        nc.tensor.matmul(out=p_all[:], lhsT=sb["w1c"][:], rhs=sb["a_row"][:],
                         start=False, stop=False)
        nc.tensor.matmul(out=p_all[:], lhsT=sb["b1r"][:],
                         rhs=sb["ones512"][:], start=False, stop=True)
        s_allT = singles.tile([K, 512], bf16, tag="allT")
        nc.vector.tensor_copy(out=s_allT[:], in_=p_all[:])

        # Z23[k, t, g, b2, b]: gate g in {2,3}, stream b2, batch-in-stream b
        s_Z23 = singles.tile([K, T, 2, 2, 2], fp32, tag="Z23")

        def precompute_z(Wpre, Wit, Wlearn, brow, g):
            ptile = pp.tile([K, T * BL], fp32, tag="pbig", bufs=1)
            nc.tensor.matmul(out=ptile[:], lhsT=sb[Wit][:],
                             rhs=sb["itT"][:, 0:T * BL], start=True, stop=False)
            nc.tensor.matmul(out=ptile[:, BL:T * BL], lhsT=sb[Wpre][:],
                             rhs=s_allT[:, 0:(T - 1) * BL],
                             start=False, stop=False, skip_group_check=True)
            nc.tensor.matmul(out=ptile[:], lhsT=sb[Wlearn][:],
                             rhs=s_allT[:, 0:T * BL], start=False, stop=False)
            nc.tensor.matmul(out=ptile[:], lhsT=sb[brow][:],
                             rhs=sb["ones512"][:, 0:T * BL], start=False,
                             stop=True)
            nc.vector.tensor_copy(
                out=s_Z23[:, :, g, :, :],
                in_=ptile[:].rearrange("k (t b2 b) -> k t b2 b", b2=2, b=2))

        precompute_z("W2a2", "W2b2", "W2c2", "b2r2", 0)
        precompute_z("W3a", "W3b", "W3c", "b3r", 1)

        # U4[k, (t,b)] = it@W4c + b4
        p_u4 = pp.tile([K, T * BL], fp32, tag="pbig", bufs=1)
        nc.tensor.matmul(out=p_u4[:], lhsT=sb["W4c"][:],
                         rhs=sb["itT"][:, 0:T * BL], start=True, stop=False)
        nc.tensor.matmul(out=p_u4[:], lhsT=sb["b4r"][:],
                         rhs=sb["ones512"][:, 0:T * BL], start=False, stop=True)
        s_U4 = singles.tile([K, T * BL], fp32, tag="U4")
        nc.vector.tensor_copy(out=s_U4[:], in_=p_u4[:])

        # Y5[k, (t,b)] = e_emb[t+1]@W5a + b5
        p_y5 = pp.tile([K, T * BL], fp32, tag="pbig", bufs=1)
        nc.tensor.matmul(out=p_y5[:], lhsT=sb["W5a"][:],
                         rhs=sb["eT"][:, BL:S * BL], start=True, stop=False)
        nc.tensor.matmul(out=p_y5[:], lhsT=sb["b5r"][:],
                         rhs=sb["ones512"][:, 0:T * BL], start=False, stop=True)
        s_Y5 = singles.tile([K, T * BL], fp32, tag="Y5")
        nc.vector.tensor_copy(out=s_Y5[:], in_=p_y5[:])

        # ---------------- h_tilde init (with q_0) ----------------
        for b in range(BL):
            cs = slice(b * C, (b + 1) * C)
            nc.vector.scalar_tensor_tensor(
                out=s_m[:, cs], in0=s_h[:, cs], scalar=0.0,
                in1=qsl(0)[:, cs], op0=OP.bypass, op1=OP.mult,
                accum_out=s_HT[:, b:b + 1])

        # ---------------- the recurrence (two 2-batch streams) ----------
        for t in range(T):
            if t % QW == 0 and t > 0 and (t // QW + 1) < NWIN:
                qwin_load(t // QW + 1)

            ps = psm.tile([K, 16], fp32, tag="small")
            for s2 in range(2):
                o = s2 * 8
                ht_sl = s_HT[:, t * BL + 2 * s2:t * BL + 2 * s2 + 2]
                nc.tensor.matmul(out=ps[:, o:o + 2], lhsT=sb["W2d2"][:],
                                 rhs=ht_sl, start=True, stop=True)
                nc.tensor.matmul(out=ps[:, o + 2:o + 4], lhsT=sb["W3d"][:],
                                 rhs=ht_sl, start=True, stop=True)
                # gamma_f preact for this stream's two batches
                pPs = pp.tile([K, 512], fp32, tag=f"pP{s2}")
                nc.tensor.matmul(out=pPs[:], lhsT=sb["W4a"][:],
                                 rhs=s_h[:, s2 * 512:(s2 + 1) * 512],
                                 start=True, stop=True)

                tAB = sm.tile([K, 4], fp32, tag=f"tAB{s2}")
                nc.vector.tensor_add(out=tAB[:], in0=ps[:, o:o + 4],
                                     in1=s_Z23[:, t, :, s2, :])
                s23 = sm.tile([K, 4], bf16, tag=f"s23{s2}")
                nc.scalar.activation(out=s23[:], in_=tAB[:], func=AF.Sigmoid)
                LGT = sm.tile([K, 2], bf16, tag=f"LGT{s2}")
                nc.gpsimd.tensor_mul(out=LGT[:], in0=s23[:, 0:2],
                                     in1=s23[:, 2:4])

                # u = LG @ W4b + U4[t]
                nc.tensor.matmul(out=ps[:, o + 4:o + 6], lhsT=sb["W4b"][:],
                                 rhs=LGT[:], start=True, stop=True)
                uT = sm.tile([K, 2], fp32, tag=f"uT{s2}")
                nc.vector.tensor_add(
                    out=uT[:], in0=ps[:, o + 4:o + 6],
                    in1=s_U4[:, t * BL + 2 * s2:t * BL + 2 * s2 + 2])

                for b in range(2):
                    gb = 2 * s2 + b
                    cs = slice(gb * C, (gb + 1) * C)
                    nc.scalar.activation(out=s_gam[:, cs],
                                         in_=pPs[:, b * C:(b + 1) * C],
                                         func=AF.Sigmoid, bias=uT[:, b:b + 1])
                    # m = gamma * h  (GpSimd)
                    nc.gpsimd.tensor_mul(out=s_m[:, cs], in0=s_gam[:, cs],
                                         in1=s_h[:, cs])
                    # h_new = q_e * LG + m
                    nc.vector.scalar_tensor_tensor(
                        out=s_h[:, cs], in0=qsl(t)[:, cs],
                        scalar=LGT[:, b:b + 1], in1=s_m[:, cs],
                        op0=OP.mult, op1=OP.add)
                    # h_tilde accumulation with q_{t+1}
                    col = (t + 1) * BL + gb
                    nc.vector.scalar_tensor_tensor(
                        out=s_m[:, cs], in0=s_h[:, cs], scalar=0.0,
                        in1=qsl(t + 1)[:, cs], op0=OP.bypass, op1=OP.mult,
                        accum_out=s_HT[:, col:col + 1])

        # ---------------- y head, batched over all steps ----------------
        p_y = pp.tile([K, T * BL], fp32, tag="pbig", bufs=1)
        nc.tensor.matmul(out=p_y[:], lhsT=sb["W5b"][:],
                         rhs=s_HT[:, BL:(T + 1) * BL], start=True, stop=True)
        tY = singles.tile([K, T * BL], fp32, tag="tY")
        nc.vector.tensor_add(out=tY[:], in0=p_y[:], in1=s_Y5[:])
        sY = singles.tile([K, T * BL], bf16, tag="sY")
        nc.scalar.activation(out=sY[:], in_=tY[:], func=AF.Sigmoid)
        p_ys = psm.tile([1, T * BL], fp32, tag="yacc", bufs=1)
        nc.tensor.matmul(out=p_ys[:], lhsT=sb["ones128c"][:], rhs=sY[:],
                         start=True, stop=True)
        s_y = singles.tile([1, T * BL], fp32, tag="yout")
        nc.vector.tensor_copy(out=s_y[:], in_=p_ys[:])
        nc.sync.dma_start(out=y_dram[:], in_=s_y[:])

    nc.compile()
    return nc


def _prep_inputs(inputs):
    """Host-side sharding + layout prep. Returns per-core input dicts."""
    import ml_dtypes

    bf = ml_dtypes.bfloat16
    f32 = np.float32
    e_idx = np.asarray(inputs["e_data"]).astype(np.int64)
    at_idx = np.asarray(inputs["at_data"]).astype(np.int64)
    it_idx = np.asarray(inputs["it_data"]).astype(np.int64)
    a_data = np.asarray(inputs["a_data"], dtype=f32)
    q_matrix = np.asarray(inputs["q_matrix"], dtype=f32)
    e_E = np.asarray(inputs["e_E"], dtype=bf)
    at_E = np.asarray(inputs["at_E"], dtype=bf)
    it_E = np.asarray(inputs["it_E"], dtype=bf)
    W1 = np.asarray(inputs["W1"], dtype=f32)
    W2 = np.asarray(inputs["W2"], dtype=f32)
    W3 = np.asarray(inputs["W3"], dtype=f32)
    W4 = np.asarray(inputs["W4"], dtype=f32)
    W5 = np.asarray(inputs["W5"], dtype=f32)
    h0 = np.asarray(inputs["h0"], dtype=f32)

    def bfc(x):
        return np.ascontiguousarray(np.asarray(x, dtype=bf))

    shared = {
        "W1a": bfc(W1[0:K]), "W1b": bfc(W1[K:2 * K]),
        "w1c": bfc(W1[2 * K:].sum(0)[None, :]),
        "b1r": bfc(np.asarray(inputs["b1"], dtype=f32)[None, :]),
        "W2a2": bfc(2 * W2[0:K]), "W2b2": bfc(2 * W2[K:2 * K]),
        "W2c2": bfc(2 * W2[2 * K:3 * K]), "W2d2": bfc(2 * W2[3 * K:]),
        "b2r2": bfc(2 * np.asarray(inputs["b2"], dtype=f32)[None, :]),
        "W3a": bfc(W3[0:K]), "W3b": bfc(W3[K:2 * K]),
        "W3c": bfc(W3[2 * K:3 * K]), "W3d": bfc(W3[3 * K:]),
        "b3r": bfc(np.asarray(inputs["b3"], dtype=f32)[None, :]),
        "W4a": bfc(W4[0:K]), "W4b": bfc(W4[K:2 * K]), "W4c": bfc(W4[2 * K:]),
        "b4r": bfc(np.asarray(inputs["b4"], dtype=f32)[None, :]),
        "W5a": bfc(W5[0:K]), "W5b": bfc(W5[K:]),
        "b5r": bfc(np.asarray(inputs["b5"], dtype=f32)[None, :]),
        "ones512": bfc(np.ones((1, 512), f32)),
        "ones128c": bfc(np.ones((K, 1), f32)),
        "h0T4": bfc(np.tile(np.ascontiguousarray(h0.T), (1, BL))),
    }

    in_maps = []
    for g in range(NCORES):
        bg = slice(g * BL, (g + 1) * BL)
        e_emb = e_E[e_idx[bg]]          # [4, S, K] bf16
        at_emb = at_E[at_idx[bg]]
        it_emb = it_E[it_idx[bg]]
        q_all = q_matrix[e_idx[bg]]     # [4, S, C] f32
        m = dict(shared)
        # [K, (s, b)] s-major layouts
        m["eT"] = bfc(e_emb.transpose(2, 1, 0).reshape(K, S * BL))
        m["atT"] = bfc(at_emb.transpose(2, 1, 0).reshape(K, S * BL))
        m["itT"] = bfc(it_emb.transpose(2, 1, 0).reshape(K, S * BL))
        m["qD"] = bfc(q_all.transpose(1, 0, 2).reshape(S, BL * C))
        m["a_row"] = bfc(a_data[bg].T.reshape(1, S * BL))
        in_maps.append(m)
    return in_maps


def _run(inputs, trace=False):
    from concourse.bass_utils import run_bass_kernel_spmd

    if "nc" not in _cache:
        _cache["nc"] = _build()
    nc = _cache["nc"]
    in_maps = _prep_inputs(inputs)
    res = run_bass_kernel_spmd(nc, in_maps, core_ids=list(range(NCORES)),
                               trace=trace)
    pred = np.zeros((B, S), np.float32)
    for g in range(NCORES):
        y = res.results[g]["y_out"].reshape(T, BL)  # [t, b]
        pred[g * BL:(g + 1) * BL, 1:] = y.T / K
    return pred, res


def kernel(**inputs):
    return _run(inputs)[0]


# revision 8
# speedup vs baseline: 2.1840x; 1.0587x over previous
"""LPKT knowledge-tracing kernel for 8x Trainium2 NeuronCores.

Data-parallel over batch: B=32 -> 4 batches per core. Per core the recurrent
state h [4, C=256, K=128] lives in SBUF as hT [K=128 partitions, (b,c)=1024
free] in bf16.  All matmuls are bf16; elementwise work is spread across DVE
(h_new, h_tilde accumulation, small adds) and GpSimd (LG gate product and
the gamma*h multiplies).  q rows are broadcast across all 128 partitions by
DMA (stride-0 source), one 16-step window (4MB) at a time, double buffered
-- the descriptors fan out over all 16 DMA engines, so no compute engine
spends cycles on replication.

The 4 batches are processed as TWO independent 2-batch streams per step so
stream B's gate chain (PE matmul -> sigmoid -> LG -> W4b matmul -> u)
overlaps stream A's state-update tail, keeping DVE/Pool busy instead of
serializing the whole step.  h_tilde columns accumulate into a persistent
HT_all buffer [K, 4*(T+1)]; gate matmuls read their [K,2] slice directly
and the y head (W5 + sigmoid + reduce) runs once, batched, after the loop.
"""

import numpy as np

B, S = 32, 128
NUM_Q, NUM_C = 10000, 256
K = 128
C = NUM_C
NCORES = 8
BL = B // NCORES  # 4 batches per core
T = S - 1  # 127 recurrence steps
QW = 16  # q broadcast window, steps
NWIN = S // QW

_cache = {}


def _build():
    import concourse.bass as bass  # noqa: F401
    import concourse.mybir as mybir
    import concourse.tile as tile
    from concourse import bacc

    fp32 = mybir.dt.float32
    bf16 = mybir.dt.bfloat16
    AF = mybir.ActivationFunctionType
    OP = mybir.AluOpType

    nc = bacc.Bacc()

    # ---------------- DRAM I/O ----------------
    d = {}

    def din(name, shape, dt_=bf16):
        t = nc.dram_tensor(name, shape, dt_, kind="ExternalInput")
        d[name] = t
        return t

    din("eT", [K, S * BL])       # e_emb^T, free layout (s, b) s-major
    din("atT", [K, S * BL])
    din("itT", [K, S * BL])
    din("a_row", [1, S * BL])
    din("h0T4", [K, BL * C])
    q_dram = nc.dram_tensor("qD", [S, BL * C], bf16, kind="ExternalInput")
    for w in ["W1a", "W1b", "W2a2", "W2b2", "W2c2", "W2d2",
              "W3a", "W3b", "W3c", "W3d", "W4a", "W4b", "W4c",
              "W5a", "W5b"]:
        din(w, [K, K])
    for w in ["w1c", "b1r", "b2r2", "b3r", "b4r", "b5r"]:
        din(w, [1, K])
    din("ones512", [1, 512])
    din("ones128c", [K, 1])
    y_dram = nc.dram_tensor("y_out", [1, BL * T], fp32, kind="ExternalOutput")

    from contextlib import ExitStack

    with tile.TileContext(nc) as tc, ExitStack() as ctx:
        singles = ctx.enter_context(tc.tile_pool(name="singles", bufs=1))
        state = ctx.enter_context(tc.tile_pool(name="state", bufs=1))
        sm = ctx.enter_context(tc.tile_pool(name="sm", bufs=3))
        qw = ctx.enter_context(tc.tile_pool(name="qw", bufs=2))
        pp = ctx.enter_context(tc.tile_pool(name="pp", bufs=2, space="PSUM"))
        psm = ctx.enter_context(tc.tile_pool(name="psm", bufs=2, space="PSUM"))

        # ---------------- load everything to SBUF ----------------
        sb = {}
        for name, dt_ in d.items():
            if name == "h0T4":
                continue  # loaded straight into the state tile below
            t_ = singles.tile(list(dt_.shape), dt_.dtype, tag=name)
            nc.sync.dma_start(out=t_[:], in_=dt_[:])
            sb[name] = t_

        # recurrent state h, DMA'd straight from the prepped h0 tile
        s_h = state.tile([K, BL * C], bf16, tag="h")
        nc.sync.dma_start(out=s_h[:], in_=d["h0T4"][:])

        # q windows: all 128 partitions get a copy of q rows [16w, 16w+16)
        qwin = [None] * NWIN

        def qwin_load(w):
            wt = qw.tile([K, QW * BL * C], bf16, tag="qwin")
            src = q_dram[w * QW:(w + 1) * QW, :].partition_broadcast(K)
            nc.sync.dma_start(out=wt[:], in_=src)
            qwin[w] = wt

        qwin_load(0)
        qwin_load(1)

        def qsl(t):
            # [K, 1024] replicated q row for step t
            base = (t % QW) * BL * C
            return qwin[t // QW][:, base:base + BL * C]

        # collapse the ~30 input-DMA dependencies
        tc.strict_bb_all_engine_barrier()

        s_gam = state.tile([K, BL * C], bf16, tag="gam")
        s_m = state.tile([K, BL * C], bf16, tag="m")
        # h_tilde history: block t (cols 4t:4t+4) = h_tilde at step t
        s_HT = state.tile([K, (T + 1) * BL], bf16, tag="HT")

        # ---------------- precompute: allT, Z23, U4, Y5 ----------------
        p_all = pp.tile([K, 512], fp32, tag="pbig", bufs=1)
        nc.tensor.matmul(out=p_all[:], lhsT=sb["W1a"][:], rhs=sb["eT"][:],
                         start=True, stop=False)
        nc.tensor.matmul(out=p_all[:], lhsT=sb["W1b"][:], rhs=sb["atT"][:],
                         start=False, stop=False)
        nc.tensor.matmul(out=p_all[:], lhsT=sb["w1c"][:], rhs=sb["a_row"][:],
                         start=False, stop=False)
        nc.tensor.matmul(out=p_all[:], lhsT=sb["b1r"][:],
                         rhs=sb["ones512"][:], start=False, stop=True)
        s_allT = singles.tile([K, 512], bf16, tag="allT")
        nc.vector.tensor_copy(out=s_allT[:], in_=p_all[:])

        # Z23[k, t, g, b2, b]: gate g in {2,3}, stream b2, batch-in-stream b
        s_Z23 = singles.tile([K, T, 2, 2, 2], fp32, tag="Z23")

        def precompute_z(Wpre, Wit, Wlearn, brow, g):
            ptile = pp.tile([K, T * BL], fp32, tag="pbig", bufs=1)
            nc.tensor.matmul(out=ptile[:], lhsT=sb[Wit][:],
                             rhs=sb["itT"][:, 0:T * BL], start=True, stop=False)
            nc.tensor.matmul(out=ptile[:, BL:T * BL], lhsT=sb[Wpre][:],
                             rhs=s_allT[:, 0:(T - 1) * BL],
                             start=False, stop=False, skip_group_check=True)
            nc.tensor.matmul(out=ptile[:], lhsT=sb[Wlearn][:],
                             rhs=s_allT[:, 0:T * BL], start=False, stop=False)
            nc.tensor.matmul(out=ptile[:], lhsT=sb[brow][:],
                             rhs=sb["ones512"][:, 0:T * BL], start=False,
                             stop=True)
            nc.vector.tensor_copy(
                out=s_Z23[:, :, g, :, :],
                in_=ptile[:].rearrange("k (t b2 b) -> k t b2 b", b2=2, b=2))

        precompute_z("W2a2", "W2b2", "W2c2", "b2r2", 0)
        precompute_z("W3a", "W3b", "W3c", "b3r", 1)

        # U4[k, (t,b)] = it@W4c + b4
        p_u4 = pp.tile([K, T * BL], fp32, tag="pbig", bufs=1)
        nc.tensor.matmul(out=p_u4[:], lhsT=sb["W4c"][:],
                         rhs=sb["itT"][:, 0:T * BL], start=True, stop=False)
        nc.tensor.matmul(out=p_u4[:], lhsT=sb["b4r"][:],
                         rhs=sb["ones512"][:, 0:T * BL], start=False, stop=True)
        s_U4 = singles.tile([K, T * BL], fp32, tag="U4")
        nc.vector.tensor_copy(out=s_U4[:], in_=p_u4[:])

        # Y5[k, (t,b)] = e_emb[t+1]@W5a + b5
        p_y5 = pp.tile([K, T * BL], fp32, tag="pbig", bufs=1)
        nc.tensor.matmul(out=p_y5[:], lhsT=sb["W5a"][:],
                         rhs=sb["eT"][:, BL:S * BL], start=True, stop=False)
        nc.tensor.matmul(out=p_y5[:], lhsT=sb["b5r"][:],
                         rhs=sb["ones512"][:, 0:T * BL], start=False, stop=True)
        s_Y5 = singles.tile([K, T * BL], fp32, tag="Y5")
        nc.vector.tensor_copy(out=s_Y5[:], in_=p_y5[:])

        # ---------------- h_tilde init (with q_0) ----------------
        for b in range(BL):
            cs = slice(b * C, (b + 1) * C)
            nc.vector.scalar_tensor_tensor(
                out=s_m[:, cs], in0=s_h[:, cs], scalar=0.0,
                in1=qsl(0)[:, cs], op0=OP.bypass, op1=OP.mult,
                accum_out=s_HT[:, b:b + 1])

        # ---------------- the recurrence (two 2-batch streams) ----------
        for t in range(T):
            if t % QW == 0 and t > 0 and (t // QW + 1) < NWIN:
                qwin_load(t // QW + 1)

            ps = psm.tile([K, 16], fp32, tag="small")
            for s2 in range(2):
                o = s2 * 8
                ht_sl = s_HT[:, t * BL + 2 * s2:t * BL + 2 * s2 + 2]
                nc.tensor.matmul(out=ps[:, o:o + 2], lhsT=sb["W2d2"][:],
                                 rhs=ht_sl, start=True, stop=True)
                nc.tensor.matmul(out=ps[:, o + 2:o + 4], lhsT=sb["W3d"][:],
                                 rhs=ht_sl, start=True, stop=True)
                # gamma_f preact for this stream's two batches
                pPs = pp.tile([K, 512], fp32, tag=f"pP{s2}")
                nc.tensor.matmul(out=pPs[:], lhsT=sb["W4a"][:],
                                 rhs=s_h[:, s2 * 512:(s2 + 1) * 512],
                                 start=True, stop=True)

                tAB = sm.tile([K, 4], fp32, tag=f"tAB{s2}")
                nc.vector.tensor_add(out=tAB[:], in0=ps[:, o:o + 4],
                                     in1=s_Z23[:, t, :, s2, :])
                s23 = sm.tile([K, 4], bf16, tag=f"s23{s2}")
                nc.scalar.activation(out=s23[:], in_=tAB[:], func=AF.Sigmoid)
                LGT = sm.tile([K, 2], bf16, tag=f"LGT{s2}")
                nc.vector.tensor_mul(out=LGT[:], in0=s23[:, 0:2],
                                     in1=s23[:, 2:4])

                # u = LG @ W4b + U4[t]
                nc.tensor.matmul(out=ps[:, o + 4:o + 6], lhsT=sb["W4b"][:],
                                 rhs=LGT[:], start=True, stop=True)
                uT = sm.tile([K, 2], fp32, tag=f"uT{s2}")
                nc.vector.tensor_add(
                    out=uT[:], in0=ps[:, o + 4:o + 6],
                    in1=s_U4[:, t * BL + 2 * s2:t * BL + 2 * s2 + 2])

                for b in range(2):
                    gb = 2 * s2 + b
                    cs = slice(gb * C, (gb + 1) * C)
                    nc.scalar.activation(out=s_gam[:, cs],
                                         in_=pPs[:, b * C:(b + 1) * C],
                                         func=AF.Sigmoid, bias=uT[:, b:b + 1])
                    # m = gamma * h
                    nc.vector.tensor_mul(out=s_m[:, cs], in0=s_gam[:, cs],
                                         in1=s_h[:, cs])
                    # h_new = q_e * LG + m
                    nc.vector.scalar_tensor_tensor(
                        out=s_h[:, cs], in0=qsl(t)[:, cs],
                        scalar=LGT[:, b:b + 1], in1=s_m[:, cs],
                        op0=OP.mult, op1=OP.add)
                    # h_tilde accumulation with q_{t+1}
                    col = (t + 1) * BL + gb
                    nc.vector.scalar_tensor_tensor(
                        out=s_m[:, cs], in0=s_h[:, cs], scalar=0.0,
                        in1=qsl(t + 1)[:, cs], op0=OP.bypass, op1=OP.mult,
                        accum_out=s_HT[:, col:col + 1])

        # ---------------- y head, batched over all steps ----------------
        p_y = pp.tile([K, T * BL], fp32, tag="pbig", bufs=1)
        nc.tensor.matmul(out=p_y[:], lhsT=sb["W5b"][:],
                         rhs=s_HT[:, BL:(T + 1) * BL], start=True, stop=True)
        tY = singles.tile([K, T * BL], fp32, tag="tY")
        nc.vector.tensor_add(out=tY[:], in0=p_y[:], in1=s_Y5[:])
        sY = singles.tile([K, T * BL], bf16, tag="sY")
        nc.scalar.activation(out=sY[:], in_=tY[:], func=AF.Sigmoid)
        p_ys = psm.tile([1, T * BL], fp32, tag="yacc", bufs=1)
        nc.tensor.matmul(out=p_ys[:], lhsT=sb["ones128c"][:], rhs=sY[:],
                         start=True, stop=True)
        s_y = singles.tile([1, T * BL], fp32, tag="yout")
        nc.vector.tensor_copy(out=s_y[:], in_=p_ys[:])
        nc.sync.dma_start(out=y_dram[:], in_=s_y[:])

    nc.compile()
    return nc


def _prep_inputs(inputs):
    """Host-side sharding + layout prep. Returns per-core input dicts."""
    import ml_dtypes

    bf = ml_dtypes.bfloat16
    f32 = np.float32
    e_idx = np.asarray(inputs["e_data"]).astype(np.int64)
    at_idx = np.asarray(inputs["at_data"]).astype(np.int64)
    it_idx = np.asarray(inputs["it_data"]).astype(np.int64)
    a_data = np.asarray(inputs["a_data"], dtype=f32)
    q_matrix = np.asarray(inputs["q_matrix"], dtype=f32)
    e_E = np.asarray(inputs["e_E"], dtype=bf)
    at_E = np.asarray(inputs["at_E"], dtype=bf)
    it_E = np.asarray(inputs["it_E"], dtype=bf)
    W1 = np.asarray(inputs["W1"], dtype=f32)
    W2 = np.asarray(inputs["W2"], dtype=f32)
    W3 = np.asarray(inputs["W3"], dtype=f32)
    W4 = np.asarray(inputs["W4"], dtype=f32)
    W5 = np.asarray(inputs["W5"], dtype=f32)
    h0 = np.asarray(inputs["h0"], dtype=f32)

    def bfc(x):
        return np.ascontiguousarray(np.asarray(x, dtype=bf))

    shared = {
        "W1a": bfc(W1[0:K]), "W1b": bfc(W1[K:2 * K]),
        "w1c": bfc(W1[2 * K:].sum(0)[None, :]),
        "b1r": bfc(np.asarray(inputs["b1"], dtype=f32)[None, :]),
        "W2a2": bfc(2 * W2[0:K]), "W2b2": bfc(2 * W2[K:2 * K]),
        "W2c2": bfc(2 * W2[2 * K:3 * K]), "W2d2": bfc(2 * W2[3 * K:]),
        "b2r2": bfc(2 * np.asarray(inputs["b2"], dtype=f32)[None, :]),
        "W3a": bfc(W3[0:K]), "W3b": bfc(W3[K:2 * K]),
        "W3c": bfc(W3[2 * K:3 * K]), "W3d": bfc(W3[3 * K:]),
        "b3r": bfc(np.asarray(inputs["b3"], dtype=f32)[None, :]),
        "W4a": bfc(W4[0:K]), "W4b": bfc(W4[K:2 * K]), "W4c": bfc(W4[2 * K:]),
        "b4r": bfc(np.asarray(inputs["b4"], dtype=f32)[None, :]),
        "W5a": bfc(W5[0:K]), "W5b": bfc(W5[K:]),
        "b5r": bfc(np.asarray(inputs["b5"], dtype=f32)[None, :]),
        "ones512": bfc(np.ones((1, 512), f32)),
        "ones128c": bfc(np.ones((K, 1), f32)),
        "h0T4": bfc(np.tile(np.ascontiguousarray(h0.T), (1, BL))),
    }

    in_maps = []
    for g in range(NCORES):
        bg = slice(g * BL, (g + 1) * BL)
        e_emb = e_E[e_idx[bg]]          # [4, S, K] bf16
        at_emb = at_E[at_idx[bg]]
        it_emb = it_E[it_idx[bg]]
        q_all = q_matrix[e_idx[bg]]     # [4, S, C] f32
        m = dict(shared)
        # [K, (s, b)] s-major layouts
        m["eT"] = bfc(e_emb.transpose(2, 1, 0).reshape(K, S * BL))
        m["atT"] = bfc(at_emb.transpose(2, 1, 0).reshape(K, S * BL))
        m["itT"] = bfc(it_emb.transpose(2, 1, 0).reshape(K, S * BL))
        m["qD"] = bfc(q_all.transpose(1, 0, 2).reshape(S, BL * C))
        m["a_row"] = bfc(a_data[bg].T.reshape(1, S * BL))
        in_maps.append(m)
    return in_maps


def _run(inputs, trace=False):
    from concourse.bass_utils import run_bass_kernel_spmd

    if "nc" not in _cache:
        _cache["nc"] = _build()
    nc = _cache["nc"]
    in_maps = _prep_inputs(inputs)
    res = run_bass_kernel_spmd(nc, in_maps, core_ids=list(range(NCORES)),
                               trace=trace)
    pred = np.zeros((B, S), np.float32)
    for g in range(NCORES):
        y = res.results[g]["y_out"].reshape(T, BL)  # [t, b]
        pred[g * BL:(g + 1) * BL, 1:] = y.T / K
    return pred, res


def kernel(**inputs):
    return _run(inputs)[0]


# revision 12
# speedup vs baseline: 2.1955x; 1.0053x over previous
"""LPKT knowledge-tracing kernel for 8x Trainium2 NeuronCores.

Data-parallel over batch: B=32 -> 4 batches per core. Per core the recurrent
state h [4, C=256, K=128] lives in SBUF as hT [K=128 partitions, (b,c)=1024
free] in bf16.  All matmuls are bf16 (1 HW pass + 1 cyc/row vs fp32's 2
passes at 4 cyc/row, and ~80ns LDWEIGHTS instead of ~440ns).  All
elementwise work is on DVE: gamma*h uses tensor_tensor (which the compiler
runs in the 2x bf16 mode, ~200ns per [128,256]); h_new and the h_tilde
accumulation need scalar_tensor_tensor (per-batch scalar / accum_out),
which only runs at 1x.  GpSimd is deliberately idle: it shares SBUF ports
with DVE, so offloading bulk elementwise there is net negative.

q rows are broadcast across all 128 partitions by DMA (stride-0 source via
AP.partition_broadcast), one 16-step window (4MB) at a time, double
buffered -- the descriptors fan out over all 16 DMA engines, so no compute
engine spends cycles on replication.

The 4 batches are processed as TWO independent 2-batch streams per step so
stream B's gate chain (PE matmul -> sigmoid -> LG -> W4b matmul -> u)
overlaps stream A's state-update tail.  h_tilde columns accumulate into a
persistent HT_all buffer [K, 4*(T+1)]; gate matmuls read their [K,2] slice
directly and the y head (W5 + sigmoid + reduce) runs once, batched, after
the loop.
"""

import numpy as np

B, S = 32, 128
NUM_Q, NUM_C = 10000, 256
K = 128
C = NUM_C
NCORES = 8
BL = B // NCORES  # 4 batches per core
T = S - 1  # 127 recurrence steps
QW = 16  # q broadcast window, steps
NWIN = S // QW

_cache = {}


def _build():
    import concourse.bass as bass  # noqa: F401
    import concourse.mybir as mybir
    import concourse.tile as tile
    from concourse import bacc

    fp32 = mybir.dt.float32
    bf16 = mybir.dt.bfloat16
    AF = mybir.ActivationFunctionType
    OP = mybir.AluOpType

    nc = bacc.Bacc()

    # ---------------- DRAM I/O ----------------
    d = {}

    def din(name, shape, dt_=bf16):
        t = nc.dram_tensor(name, shape, dt_, kind="ExternalInput")
        d[name] = t
        return t

    din("eT", [K, S * BL])       # e_emb^T, free layout (s, b) s-major
    din("atT", [K, S * BL])
    din("itT", [K, S * BL])
    din("a_row", [1, S * BL])
    din("h0T4", [K, BL * C])
    q_dram = nc.dram_tensor("qD", [S, BL * C], bf16, kind="ExternalInput")
    for w in ["W1a", "W1b", "W2a2", "W2b2", "W2c2", "W2d2",
              "W3a", "W3b", "W3c", "W3d", "W4a", "W4b", "W4c",
              "W5a", "W5b"]:
        din(w, [K, K])
    for w in ["w1c", "b1r", "b2r2", "b3r", "b4r", "b5r"]:
        din(w, [1, K])
    din("ones512", [1, 512])
    din("I128", [K, K])
    din("ones128c", [K, 1])
    y_dram = nc.dram_tensor("y_out", [1, BL * T], fp32, kind="ExternalOutput")

    from contextlib import ExitStack

    with tile.TileContext(nc) as tc, ExitStack() as ctx:
        singles = ctx.enter_context(tc.tile_pool(name="singles", bufs=1))
        state = ctx.enter_context(tc.tile_pool(name="state", bufs=1))
        sm = ctx.enter_context(tc.tile_pool(name="sm", bufs=3))
        qw = ctx.enter_context(tc.tile_pool(name="qw", bufs=2))
        pp = ctx.enter_context(tc.tile_pool(name="pp", bufs=2, space="PSUM"))
        psm = ctx.enter_context(tc.tile_pool(name="psm", bufs=2, space="PSUM"))

        # ---------------- load everything to SBUF ----------------
        sb = {}
        for name, dt_ in d.items():
            if name == "h0T4":
                continue  # loaded straight into the state tile below
            t_ = singles.tile(list(dt_.shape), dt_.dtype, tag=name)
            nc.sync.dma_start(out=t_[:], in_=dt_[:])
            sb[name] = t_

        # recurrent state h, DMA'd straight from the prepped h0 tile
        s_h = state.tile([K, BL * C], bf16, tag="h")
        nc.sync.dma_start(out=s_h[:], in_=d["h0T4"][:])

        # q windows: all 128 partitions get a copy of q rows [16w, 16w+16)
        qwin = [None] * NWIN

        def qwin_load(w):
            wt = qw.tile([K, QW * BL * C], bf16, tag="qwin")
            src = q_dram[w * QW:(w + 1) * QW, :].partition_broadcast(K)
            nc.sync.dma_start(out=wt[:], in_=src)
            qwin[w] = wt

        qwin_load(0)
        qwin_load(1)

        def qsl(t):
            # [K, 1024] replicated q row for step t
            base = (t % QW) * BL * C
            return qwin[t // QW][:, base:base + BL * C]

        # collapse the ~30 input-DMA dependencies
        tc.strict_bb_all_engine_barrier()

        s_gam = state.tile([K, BL * C], bf16, tag="gam")
        s_m = state.tile([K, BL * C], bf16, tag="m")
        # h_tilde history: block t (cols 4t:4t+4) = h_tilde at step t
        s_HT = state.tile([K, (T + 1) * BL], bf16, tag="HT")

        # ---------------- precompute: allT, Z23, U4, Y5 ----------------
        p_all = pp.tile([K, 512], fp32, tag="pbig", bufs=1)
        nc.tensor.matmul(out=p_all[:], lhsT=sb["W1a"][:], rhs=sb["eT"][:],
                         start=True, stop=False)
        nc.tensor.matmul(out=p_all[:], lhsT=sb["W1b"][:], rhs=sb["atT"][:],
                         start=False, stop=False)
        nc.tensor.matmul(out=p_all[:], lhsT=sb["w1c"][:], rhs=sb["a_row"][:],
                         start=False, stop=False)
        nc.tensor.matmul(out=p_all[:], lhsT=sb["b1r"][:],
                         rhs=sb["ones512"][:], start=False, stop=True)
        s_allT = singles.tile([K, 512], bf16, tag="allT")
        nc.vector.tensor_copy(out=s_allT[:], in_=p_all[:])

        # Z23[k, t, g, b2, b]: gate g in {2,3}, stream b2, batch-in-stream b
        s_Z23 = singles.tile([K, T, 2, 2, 2], fp32, tag="Z23")

        def precompute_z(Wpre, Wit, Wlearn, brow, g):
            ptile = pp.tile([K, T * BL], fp32, tag="pbig", bufs=1)
            nc.tensor.matmul(out=ptile[:], lhsT=sb[Wit][:],
                             rhs=sb["itT"][:, 0:T * BL], start=True, stop=False)
            nc.tensor.matmul(out=ptile[:, BL:T * BL], lhsT=sb[Wpre][:],
                             rhs=s_allT[:, 0:(T - 1) * BL],
                             start=False, stop=False, skip_group_check=True)
            nc.tensor.matmul(out=ptile[:], lhsT=sb[Wlearn][:],
                             rhs=s_allT[:, 0:T * BL], start=False, stop=False)
            nc.tensor.matmul(out=ptile[:], lhsT=sb[brow][:],
                             rhs=sb["ones512"][:, 0:T * BL], start=False,
                             stop=True)
            nc.vector.tensor_copy(
                out=s_Z23[:, :, g, :, :],
                in_=ptile[:].rearrange("k (t b2 b) -> k t b2 b", b2=2, b=2))

        precompute_z("W2a2", "W2b2", "W2c2", "b2r2", 0)
        precompute_z("W3a", "W3b", "W3c", "b3r", 1)

        # U4[k, (t,b)] = it@W4c + b4
        p_u4 = pp.tile([K, T * BL], fp32, tag="pbig", bufs=1)
        nc.tensor.matmul(out=p_u4[:], lhsT=sb["W4c"][:],
                         rhs=sb["itT"][:, 0:T * BL], start=True, stop=False)
        nc.tensor.matmul(out=p_u4[:], lhsT=sb["b4r"][:],
                         rhs=sb["ones512"][:, 0:T * BL], start=False, stop=True)
        s_U4 = singles.tile([K, T * BL], fp32, tag="U4")
        nc.vector.tensor_copy(out=s_U4[:], in_=p_u4[:])

        # Y5[k, (t,b)] = e_emb[t+1]@W5a + b5
        p_y5 = pp.tile([K, T * BL], fp32, tag="pbig", bufs=1)
        nc.tensor.matmul(out=p_y5[:], lhsT=sb["W5a"][:],
                         rhs=sb["eT"][:, BL:S * BL], start=True, stop=False)
        nc.tensor.matmul(out=p_y5[:], lhsT=sb["b5r"][:],
                         rhs=sb["ones512"][:, 0:T * BL], start=False, stop=True)
        s_Y5 = singles.tile([K, T * BL], fp32, tag="Y5")
        nc.vector.tensor_copy(out=s_Y5[:], in_=p_y5[:])

        # ---------------- h_tilde init (with q_0) ----------------
        for b in range(BL):
            cs = slice(b * C, (b + 1) * C)
            nc.vector.scalar_tensor_tensor(
                out=s_m[:, cs], in0=s_h[:, cs], scalar=0.0,
                in1=qsl(0)[:, cs], op0=OP.bypass, op1=OP.mult,
                accum_out=s_HT[:, b:b + 1])

        # ---------------- the recurrence (two 2-batch streams) ----------
        for t in range(T):
            if t % QW == 0 and t > 0 and (t // QW + 1) < NWIN:
                qwin_load(t // QW + 1)

            ps = psm.tile([K, 16], fp32, tag="small")
            for s2 in range(2):
                o = s2 * 8
                ht_sl = s_HT[:, t * BL + 2 * s2:t * BL + 2 * s2 + 2]
                nc.tensor.matmul(out=ps[:, o:o + 2], lhsT=sb["W2d2"][:],
                                 rhs=ht_sl, start=True, stop=True)
                nc.tensor.matmul(out=ps[:, o + 2:o + 4], lhsT=sb["W3d"][:],
                                 rhs=ht_sl, start=True, stop=True)
                # gamma_f preact for this stream's two batches
                pPs = pp.tile([K, 512], fp32, tag=f"pP{s2}")
                nc.tensor.matmul(out=pPs[:], lhsT=sb["W4a"][:],
                                 rhs=s_h[:, s2 * 512:(s2 + 1) * 512],
                                 start=True, stop=True)

                tAB = sm.tile([K, 4], fp32, tag=f"tAB{s2}")
                nc.vector.tensor_add(out=tAB[:], in0=ps[:, o:o + 4],
                                     in1=s_Z23[:, t, :, s2, :])
                s23 = sm.tile([K, 4], bf16, tag=f"s23{s2}")
                nc.scalar.activation(out=s23[:], in_=tAB[:], func=AF.Sigmoid)
                LGT = sm.tile([K, 2], bf16, tag=f"LGT{s2}")
                nc.vector.tensor_mul(out=LGT[:], in0=s23[:, 0:2],
                                     in1=s23[:, 2:4])

                # u = LG @ W4b + U4[t]
                nc.tensor.matmul(out=ps[:, o + 4:o + 6], lhsT=sb["W4b"][:],
                                 rhs=LGT[:], start=True, stop=True)
                uT = sm.tile([K, 2], fp32, tag=f"uT{s2}")
                nc.vector.tensor_add(
                    out=uT[:], in0=ps[:, o + 4:o + 6],
                    in1=s_U4[:, t * BL + 2 * s2:t * BL + 2 * s2 + 2])

                for b in range(2):
                    gb = 2 * s2 + b
                    cs = slice(gb * C, (gb + 1) * C)
                    nc.scalar.activation(out=s_gam[:, cs],
                                         in_=pPs[:, b * C:(b + 1) * C],
                                         func=AF.Sigmoid, bias=uT[:, b:b + 1])
                    # m = gamma * h
                    nc.vector.tensor_mul(out=s_m[:, cs], in0=s_gam[:, cs],
                                         in1=s_h[:, cs])
                    # h_new = q_e * LG + m
                    nc.vector.scalar_tensor_tensor(
                        out=s_h[:, cs], in0=qsl(t)[:, cs],
                        scalar=LGT[:, b:b + 1], in1=s_m[:, cs],
                        op0=OP.mult, op1=OP.add)
                    # h_tilde accumulation with q_{t+1}
                    col = (t + 1) * BL + gb
                    nc.vector.scalar_tensor_tensor(
                        out=s_m[:, cs], in0=s_h[:, cs], scalar=0.0,
                        in1=qsl(t + 1)[:, cs], op0=OP.bypass, op1=OP.mult,
                        accum_out=s_HT[:, col:col + 1])

        # ---------------- y head, batched over all steps ----------------
        p_y = pp.tile([K, T * BL], fp32, tag="pbig", bufs=1)
        nc.tensor.matmul(out=p_y[:], lhsT=sb["W5b"][:],
                         rhs=s_HT[:, BL:(T + 1) * BL], start=True, stop=True)
        tY = singles.tile([K, T * BL], fp32, tag="tY")
        nc.vector.tensor_add(out=tY[:], in0=p_y[:], in1=s_Y5[:])
        sY = singles.tile([K, T * BL], bf16, tag="sY")
        nc.scalar.activation(out=sY[:], in_=tY[:], func=AF.Sigmoid)
        p_ys = psm.tile([1, T * BL], fp32, tag="yacc", bufs=1)
        nc.tensor.matmul(out=p_ys[:], lhsT=sb["ones128c"][:], rhs=sY[:],
                         start=True, stop=True)
        s_y = singles.tile([1, T * BL], fp32, tag="yout")
        nc.vector.tensor_copy(out=s_y[:], in_=p_ys[:])
        nc.sync.dma_start(out=y_dram[:], in_=s_y[:])

    nc.compile()
    return nc


def _prep_inputs(inputs):
    """Host-side sharding + layout prep. Returns per-core input dicts."""
    import ml_dtypes

    bf = ml_dtypes.bfloat16
    f32 = np.float32
    e_idx = np.asarray(inputs["e_data"]).astype(np.int64)
    at_idx = np.asarray(inputs["at_data"]).astype(np.int64)
    it_idx = np.asarray(inputs["it_data"]).astype(np.int64)
    a_data = np.asarray(inputs["a_data"], dtype=f32)
    q_matrix = np.asarray(inputs["q_matrix"], dtype=f32)
    e_E = np.asarray(inputs["e_E"], dtype=bf)
    at_E = np.asarray(inputs["at_E"], dtype=bf)
    it_E = np.asarray(inputs["it_E"], dtype=bf)
    W1 = np.asarray(inputs["W1"], dtype=f32)
    W2 = np.asarray(inputs["W2"], dtype=f32)
    W3 = np.asarray(inputs["W3"], dtype=f32)
    W4 = np.asarray(inputs["W4"], dtype=f32)
    W5 = np.asarray(inputs["W5"], dtype=f32)
    h0 = np.asarray(inputs["h0"], dtype=f32)

    def bfc(x):
        return np.ascontiguousarray(np.asarray(x, dtype=bf))

    shared = {
        "W1a": bfc(W1[0:K]), "W1b": bfc(W1[K:2 * K]),
        "w1c": bfc(W1[2 * K:].sum(0)[None, :]),
        "b1r": bfc(np.asarray(inputs["b1"], dtype=f32)[None, :]),
        "W2a2": bfc(2 * W2[0:K]), "W2b2": bfc(2 * W2[K:2 * K]),
        "W2c2": bfc(2 * W2[2 * K:3 * K]), "W2d2": bfc(2 * W2[3 * K:]),
        "b2r2": bfc(2 * np.asarray(inputs["b2"], dtype=f32)[None, :]),
        "W3a": bfc(W3[0:K]), "W3b": bfc(W3[K:2 * K]),
        "W3c": bfc(W3[2 * K:3 * K]), "W3d": bfc(W3[3 * K:]),
        "b3r": bfc(np.asarray(inputs["b3"], dtype=f32)[None, :]),
        "W4a": bfc(W4[0:K]), "W4b": bfc(W4[K:2 * K]), "W4c": bfc(W4[2 * K:]),
        "b4r": bfc(np.asarray(inputs["b4"], dtype=f32)[None, :]),
        "W5a": bfc(W5[0:K]), "W5b": bfc(W5[K:]),
        "b5r": bfc(np.asarray(inputs["b5"], dtype=f32)[None, :]),
        "ones512": bfc(np.ones((1, 512), f32)),
        "ones128c": bfc(np.ones((K, 1), f32)),
        "I128": bfc(np.eye(K, dtype=f32)),
        "h0T4": bfc(np.tile(np.ascontiguousarray(h0.T), (1, BL))),
    }

    in_maps = []
    for g in range(NCORES):
        bg = slice(g * BL, (g + 1) * BL)
        e_emb = e_E[e_idx[bg]]          # [4, S, K] bf16
        at_emb = at_E[at_idx[bg]]
        it_emb = it_E[it_idx[bg]]
        q_all = q_matrix[e_idx[bg]]     # [4, S, C] f32
        m = dict(shared)
        # [K, (s, b)] s-major layouts
        m["eT"] = bfc(e_emb.transpose(2, 1, 0).reshape(K, S * BL))
        m["atT"] = bfc(at_emb.transpose(2, 1, 0).reshape(K, S * BL))
        m["itT"] = bfc(it_emb.transpose(2, 1, 0).reshape(K, S * BL))
        m["qD"] = bfc(q_all.transpose(1, 0, 2).reshape(S, BL * C))
        m["a_row"] = bfc(a_data[bg].T.reshape(1, S * BL))
        in_maps.append(m)
    return in_maps


def _run(inputs, trace=False):
    from concourse.bass_utils import run_bass_kernel_spmd

    if "nc" not in _cache:
        _cache["nc"] = _build()
    nc = _cache["nc"]
    in_maps = _prep_inputs(inputs)
    res = run_bass_kernel_spmd(nc, in_maps, core_ids=list(range(NCORES)),
                               trace=trace)
    pred = np.zeros((B, S), np.float32)
    for g in range(NCORES):
        y = res.results[g]["y_out"].reshape(T, BL)  # [t, b]
        pred[g * BL:(g + 1) * BL, 1:] = y.T / K
    return pred, res


def kernel(**inputs):
    return _run(inputs)[0]


# revision 13
# speedup vs baseline: 2.5898x; 1.1796x over previous
"""LPKT knowledge-tracing kernel for 8x Trainium2 NeuronCores.

Data-parallel over batch: B=32 -> 4 batches per core. Per core the recurrent
state h [4, C=256, K=128] lives in SBUF as hT [K=128 partitions, (b,c)=1024
free] in bf16.  All matmuls are bf16 (1 HW pass + 1 cyc/row vs fp32's 2
passes at 4 cyc/row, and ~80ns LDWEIGHTS instead of ~440ns).  All
elementwise work is on DVE: gamma*h uses tensor_tensor (which the compiler
runs in the 2x bf16 mode, ~200ns per [128,256]); h_new and the h_tilde
accumulation need scalar_tensor_tensor (per-batch scalar / accum_out),
which only runs at 1x.  GpSimd is deliberately idle: it shares SBUF ports
with DVE, so offloading bulk elementwise there is net negative.

q rows are broadcast across all 128 partitions by DMA (stride-0 source via
AP.partition_broadcast), one 16-step window (4MB) at a time, double
buffered -- the descriptors fan out over all 16 DMA engines, so no compute
engine spends cycles on replication.

The 4 batches are processed as TWO independent 2-batch streams per step so
stream B's gate chain (PE matmul -> sigmoid -> LG -> W4b matmul -> u)
overlaps stream A's state-update tail.  h_tilde columns accumulate into a
persistent HT_all buffer [K, 4*(T+1)]; gate matmuls read their [K,2] slice
directly and the y head (W5 + sigmoid + reduce) runs once, batched, after
the loop.
"""

import numpy as np

B, S = 32, 128
NUM_Q, NUM_C = 10000, 256
K = 128
C = NUM_C
NCORES = 8
BL = B // NCORES  # 4 batches per core
T = S - 1  # 127 recurrence steps
QW = 16  # q broadcast window, steps
NWIN = S // QW

_cache = {}


def _build():
    import concourse.bass as bass  # noqa: F401
    import concourse.mybir as mybir
    import concourse.tile as tile
    from concourse import bacc

    fp32 = mybir.dt.float32
    bf16 = mybir.dt.bfloat16
    AF = mybir.ActivationFunctionType
    OP = mybir.AluOpType

    nc = bacc.Bacc()

    # ---------------- DRAM I/O ----------------
    d = {}

    def din(name, shape, dt_=bf16):
        t = nc.dram_tensor(name, shape, dt_, kind="ExternalInput")
        d[name] = t
        return t

    din("eT", [K, S * BL])       # e_emb^T, free layout (s, b) s-major
    din("atT", [K, S * BL])
    din("itT", [K, S * BL])
    din("a_row", [1, S * BL])
    din("h0T4", [K, BL * C])
    q_dram = nc.dram_tensor("qD", [S, BL * C], bf16, kind="ExternalInput")
    for w in ["W1a", "W1b", "W2a2", "W2b2", "W2c2", "W2d2",
              "W3a", "W3b", "W3c", "W3d", "W4a", "W4b", "W4c",
              "W5a", "W5b"]:
        din(w, [K, K])
    for w in ["w1c", "b1r", "b2r2", "b3r", "b4r", "b5r"]:
        din(w, [1, K])
    din("ones512", [1, 512])
    din("I128", [K, K])
    din("ones128c", [K, 1])
    y_dram = nc.dram_tensor("y_out", [1, BL * T], fp32, kind="ExternalOutput")

    from contextlib import ExitStack

    with tile.TileContext(nc) as tc, ExitStack() as ctx:
        singles = ctx.enter_context(tc.tile_pool(name="singles", bufs=1))
        state = ctx.enter_context(tc.tile_pool(name="state", bufs=1))
        sm = ctx.enter_context(tc.tile_pool(name="sm", bufs=3))
        qw = ctx.enter_context(tc.tile_pool(name="qw", bufs=2))
        pp = ctx.enter_context(tc.tile_pool(name="pp", bufs=2, space="PSUM"))
        psm = ctx.enter_context(tc.tile_pool(name="psm", bufs=2, space="PSUM"))

        # ---------------- load everything to SBUF ----------------
        sb = {}
        for name, dt_ in d.items():
            if name == "h0T4":
                continue  # loaded straight into the state tile below
            t_ = singles.tile(list(dt_.shape), dt_.dtype, tag=name)
            nc.sync.dma_start(out=t_[:], in_=dt_[:])
            sb[name] = t_

        # recurrent state h, DMA'd straight from the prepped h0 tile
        s_h = state.tile([K, BL * C], bf16, tag="h")
        nc.sync.dma_start(out=s_h[:], in_=d["h0T4"][:])

        # q windows: all 128 partitions get a copy of q rows [16w, 16w+16)
        qwin = [None] * NWIN

        def qwin_load(w):
            wt = qw.tile([K, QW * BL * C], bf16, tag="qwin")
            src = q_dram[w * QW:(w + 1) * QW, :].partition_broadcast(K)
            nc.sync.dma_start(out=wt[:], in_=src)
            qwin[w] = wt

        qwin_load(0)
        qwin_load(1)

        def qsl(t):
            # [K, 1024] replicated q row for step t
            base = (t % QW) * BL * C
            return qwin[t // QW][:, base:base + BL * C]

        # collapse the ~30 input-DMA dependencies
        tc.strict_bb_all_engine_barrier()

        s_gam = state.tile([K, BL * C], bf16, tag="gam")
        s_m = state.tile([K, BL * C], bf16, tag="m")
        # h_tilde history: block t (cols 4t:4t+4) = h_tilde at step t
        s_HT = state.tile([K, (T + 1) * BL], bf16, tag="HT")

        # ---------------- precompute: allT, Z23, U4, Y5 ----------------
        p_all = pp.tile([K, 512], fp32, tag="pbig", bufs=1)
        nc.tensor.matmul(out=p_all[:], lhsT=sb["W1a"][:], rhs=sb["eT"][:],
                         start=True, stop=False)
        nc.tensor.matmul(out=p_all[:], lhsT=sb["W1b"][:], rhs=sb["atT"][:],
                         start=False, stop=False)
        nc.tensor.matmul(out=p_all[:], lhsT=sb["w1c"][:], rhs=sb["a_row"][:],
                         start=False, stop=False)
        nc.tensor.matmul(out=p_all[:], lhsT=sb["b1r"][:],
                         rhs=sb["ones512"][:], start=False, stop=True)
        s_allT = singles.tile([K, 512], bf16, tag="allT")
        nc.vector.tensor_copy(out=s_allT[:], in_=p_all[:])

        # Z23[k, t, g, b2, b]: gate g in {2,3}, stream b2, batch-in-stream b
        s_Z23 = singles.tile([K, T, 2, 2, 2], bf16, tag="Z23")

        def precompute_z(Wpre, Wit, Wlearn, brow, g):
            ptile = pp.tile([K, T * BL], fp32, tag="pbig", bufs=1)
            nc.tensor.matmul(out=ptile[:], lhsT=sb[Wit][:],
                             rhs=sb["itT"][:, 0:T * BL], start=True, stop=False)
            nc.tensor.matmul(out=ptile[:, BL:T * BL], lhsT=sb[Wpre][:],
                             rhs=s_allT[:, 0:(T - 1) * BL],
                             start=False, stop=False, skip_group_check=True)
            nc.tensor.matmul(out=ptile[:], lhsT=sb[Wlearn][:],
                             rhs=s_allT[:, 0:T * BL], start=False, stop=False)
            nc.tensor.matmul(out=ptile[:], lhsT=sb[brow][:],
                             rhs=sb["ones512"][:, 0:T * BL], start=False,
                             stop=True)
            nc.vector.tensor_copy(
                out=s_Z23[:, :, g, :, :],
                in_=ptile[:].rearrange("k (t b2 b) -> k t b2 b", b2=2, b=2))

        precompute_z("W2a2", "W2b2", "W2c2", "b2r2", 0)
        precompute_z("W3a", "W3b", "W3c", "b3r", 1)

        # U4[k, (t,b)] = it@W4c + b4
        p_u4 = pp.tile([K, T * BL], fp32, tag="pbig", bufs=1)
        nc.tensor.matmul(out=p_u4[:], lhsT=sb["W4c"][:],
                         rhs=sb["itT"][:, 0:T * BL], start=True, stop=False)
        nc.tensor.matmul(out=p_u4[:], lhsT=sb["b4r"][:],
                         rhs=sb["ones512"][:, 0:T * BL], start=False, stop=True)
        s_U4 = singles.tile([K, T * BL], bf16, tag="U4")
        nc.vector.tensor_copy(out=s_U4[:], in_=p_u4[:])

        # Y5[k, (t,b)] = e_emb[t+1]@W5a + b5
        p_y5 = pp.tile([K, T * BL], fp32, tag="pbig", bufs=1)
        nc.tensor.matmul(out=p_y5[:], lhsT=sb["W5a"][:],
                         rhs=sb["eT"][:, BL:S * BL], start=True, stop=False)
        nc.tensor.matmul(out=p_y5[:], lhsT=sb["b5r"][:],
                         rhs=sb["ones512"][:, 0:T * BL], start=False, stop=True)
        s_Y5 = singles.tile([K, T * BL], fp32, tag="Y5")
        nc.vector.tensor_copy(out=s_Y5[:], in_=p_y5[:])

        # ---------------- h_tilde init (with q_0) ----------------
        for b in range(BL):
            cs = slice(b * C, (b + 1) * C)
            nc.vector.scalar_tensor_tensor(
                out=s_m[:, cs], in0=s_h[:, cs], scalar=0.0,
                in1=qsl(0)[:, cs], op0=OP.bypass, op1=OP.mult,
                accum_out=s_HT[:, b:b + 1])

        # ---------------- the recurrence (two 2-batch streams) ----------
        for t in range(T):
            if t % QW == 0 and t > 0 and (t // QW + 1) < NWIN:
                qwin_load(t // QW + 1)

            ps = psm.tile([K, 16], fp32, tag="small")
            for s2 in range(2):
                o = s2 * 8
                ht_sl = s_HT[:, t * BL + 2 * s2:t * BL + 2 * s2 + 2]
                nc.tensor.matmul(out=ps[:, o:o + 2], lhsT=sb["W2d2"][:],
                                 rhs=ht_sl, start=True, stop=False)
                nc.tensor.matmul(out=ps[:, o + 2:o + 4], lhsT=sb["W3d"][:],
                                 rhs=ht_sl, start=True, stop=False)
                # += Z23[t] on PE (identity pass-through), so no DVE add sits
                # on the gate-critical path
                nc.tensor.matmul(out=ps[:, o:o + 4], lhsT=sb["I128"][:],
                                 rhs=s_Z23[:, t, :, s2, :], start=False,
                                 stop=True, skip_group_check=True)
                # gamma_f preact for this stream's two batches
                pPs = pp.tile([K, 512], fp32, tag=f"pP{s2}")
                nc.tensor.matmul(out=pPs[:], lhsT=sb["W4a"][:],
                                 rhs=s_h[:, s2 * 512:(s2 + 1) * 512],
                                 start=True, stop=True)

                s23 = sm.tile([K, 4], bf16, tag=f"s23{s2}")
                nc.scalar.activation(out=s23[:], in_=ps[:, o:o + 4],
                                     func=AF.Sigmoid)
                # LG on the otherwise-idle GpSimd queue: never waits behind
                # DVE bulk work
                LGT = sm.tile([K, 2], bf16, tag=f"LGT{s2}")
                nc.gpsimd.tensor_mul(out=LGT[:], in0=s23[:, 0:2],
                                     in1=s23[:, 2:4])

                # u = LG @ W4b + U4[t] (U4 added on PE); psum -> SBUF move on
                # ACT, same queue as the gamma sigmoids that consume it
                nc.tensor.matmul(out=ps[:, o + 4:o + 6], lhsT=sb["W4b"][:],
                                 rhs=LGT[:], start=True, stop=False)
                nc.tensor.matmul(
                    out=ps[:, o + 4:o + 6], lhsT=sb["I128"][:],
                    rhs=s_U4[:, t * BL + 2 * s2:t * BL + 2 * s2 + 2],
                    start=False, stop=True)
                uT = sm.tile([K, 2], fp32, tag=f"uT{s2}")
                nc.scalar.copy(out=uT[:], in_=ps[:, o + 4:o + 6])

                for b in range(2):
                    gb = 2 * s2 + b
                    cs = slice(gb * C, (gb + 1) * C)
                    nc.scalar.activation(out=s_gam[:, cs],
                                         in_=pPs[:, b * C:(b + 1) * C],
                                         func=AF.Sigmoid, bias=uT[:, b:b + 1])
                    # m = gamma * h
                    nc.vector.tensor_mul(out=s_m[:, cs], in0=s_gam[:, cs],
                                         in1=s_h[:, cs])
                    # h_new = q_e * LG + m
                    nc.vector.scalar_tensor_tensor(
                        out=s_h[:, cs], in0=qsl(t)[:, cs],
                        scalar=LGT[:, b:b + 1], in1=s_m[:, cs],
                        op0=OP.mult, op1=OP.add)
                    # h_tilde accumulation with q_{t+1}
                    col = (t + 1) * BL + gb
                    nc.vector.scalar_tensor_tensor(
                        out=s_m[:, cs], in0=s_h[:, cs], scalar=0.0,
                        in1=qsl(t + 1)[:, cs], op0=OP.bypass, op1=OP.mult,
                        accum_out=s_HT[:, col:col + 1])

        # ---------------- y head, batched over all steps ----------------
        p_y = pp.tile([K, T * BL], fp32, tag="pbig", bufs=1)
        nc.tensor.matmul(out=p_y[:], lhsT=sb["W5b"][:],
                         rhs=s_HT[:, BL:(T + 1) * BL], start=True, stop=True)
        tY = singles.tile([K, T * BL], fp32, tag="tY")
        nc.vector.tensor_add(out=tY[:], in0=p_y[:], in1=s_Y5[:])
        sY = singles.tile([K, T * BL], bf16, tag="sY")
        nc.scalar.activation(out=sY[:], in_=tY[:], func=AF.Sigmoid)
        p_ys = psm.tile([1, T * BL], fp32, tag="yacc", bufs=1)
        nc.tensor.matmul(out=p_ys[:], lhsT=sb["ones128c"][:], rhs=sY[:],
                         start=True, stop=True)
        s_y = singles.tile([1, T * BL], fp32, tag="yout")
        nc.vector.tensor_copy(out=s_y[:], in_=p_ys[:])
        nc.sync.dma_start(out=y_dram[:], in_=s_y[:])

    nc.compile()
    return nc


def _prep_inputs(inputs):
    """Host-side sharding + layout prep. Returns per-core input dicts."""
    import ml_dtypes

    bf = ml_dtypes.bfloat16
    f32 = np.float32
    e_idx = np.asarray(inputs["e_data"]).astype(np.int64)
    at_idx = np.asarray(inputs["at_data"]).astype(np.int64)
    it_idx = np.asarray(inputs["it_data"]).astype(np.int64)
    a_data = np.asarray(inputs["a_data"], dtype=f32)
    q_matrix = np.asarray(inputs["q_matrix"], dtype=f32)
    e_E = np.asarray(inputs["e_E"], dtype=bf)
    at_E = np.asarray(inputs["at_E"], dtype=bf)
    it_E = np.asarray(inputs["it_E"], dtype=bf)
    W1 = np.asarray(inputs["W1"], dtype=f32)
    W2 = np.asarray(inputs["W2"], dtype=f32)
    W3 = np.asarray(inputs["W3"], dtype=f32)
    W4 = np.asarray(inputs["W4"], dtype=f32)
    W5 = np.asarray(inputs["W5"], dtype=f32)
    h0 = np.asarray(inputs["h0"], dtype=f32)

    def bfc(x):
        return np.ascontiguousarray(np.asarray(x, dtype=bf))

    shared = {
        "W1a": bfc(W1[0:K]), "W1b": bfc(W1[K:2 * K]),
        "w1c": bfc(W1[2 * K:].sum(0)[None, :]),
        "b1r": bfc(np.asarray(inputs["b1"], dtype=f32)[None, :]),
        "W2a2": bfc(2 * W2[0:K]), "W2b2": bfc(2 * W2[K:2 * K]),
        "W2c2": bfc(2 * W2[2 * K:3 * K]), "W2d2": bfc(2 * W2[3 * K:]),
        "b2r2": bfc(2 * np.asarray(inputs["b2"], dtype=f32)[None, :]),
        "W3a": bfc(W3[0:K]), "W3b": bfc(W3[K:2 * K]),
        "W3c": bfc(W3[2 * K:3 * K]), "W3d": bfc(W3[3 * K:]),
        "b3r": bfc(np.asarray(inputs["b3"], dtype=f32)[None, :]),
        "W4a": bfc(W4[0:K]), "W4b": bfc(W4[K:2 * K]), "W4c": bfc(W4[2 * K:]),
        "b4r": bfc(np.asarray(inputs["b4"], dtype=f32)[None, :]),
        "W5a": bfc(W5[0:K]), "W5b": bfc(W5[K:]),
        "b5r": bfc(np.asarray(inputs["b5"], dtype=f32)[None, :]),
        "ones512": bfc(np.ones((1, 512), f32)),
        "ones128c": bfc(np.ones((K, 1), f32)),
        "I128": bfc(np.eye(K, dtype=f32)),
        "h0T4": bfc(np.tile(np.ascontiguousarray(h0.T), (1, BL))),
    }

    in_maps = []
    for g in range(NCORES):
        bg = slice(g * BL, (g + 1) * BL)
        e_emb = e_E[e_idx[bg]]          # [4, S, K] bf16
        at_emb = at_E[at_idx[bg]]
        it_emb = it_E[it_idx[bg]]
        q_all = q_matrix[e_idx[bg]]     # [4, S, C] f32
        m = dict(shared)
        # [K, (s, b)] s-major layouts
        m["eT"] = bfc(e_emb.transpose(2, 1, 0).reshape(K, S * BL))
        m["atT"] = bfc(at_emb.transpose(2, 1, 0).reshape(K, S * BL))
        m["itT"] = bfc(it_emb.transpose(2, 1, 0).reshape(K, S * BL))
        m["qD"] = bfc(q_all.transpose(1, 0, 2).reshape(S, BL * C))
        m["a_row"] = bfc(a_data[bg].T.reshape(1, S * BL))
        in_maps.append(m)
    return in_maps


def _run(inputs, trace=False):
    from concourse.bass_utils import run_bass_kernel_spmd

    if "nc" not in _cache:
        _cache["nc"] = _build()
    nc = _cache["nc"]
    in_maps = _prep_inputs(inputs)
    res = run_bass_kernel_spmd(nc, in_maps, core_ids=list(range(NCORES)),
                               trace=trace)
    pred = np.zeros((B, S), np.float32)
    for g in range(NCORES):
        y = res.results[g]["y_out"].reshape(T, BL)  # [t, b]
        pred[g * BL:(g + 1) * BL, 1:] = y.T / K
    return pred, res


def kernel(**inputs):
    return _run(inputs)[0]
